# revision 1
# baseline (speedup 1.0000x reference)
"""GQA attention kernel for 8 Trainium2 NeuronCores.

Sharding: core = (batch b, kv_group g), b in {0,1}, g in {0..3}.
Each core computes the 4 heads of one KV group for one batch and the
partial output projection for those heads; the host sums the 4 group
partials per batch.  Zero duplicated compute across cores.

Per-core layout choices (all matmuls run in float32r = full PE rate):
  - host passes xT = x[b].T so every projection has contraction on
    partitions without any on-device transpose of x
  - QT/KT are produced directly in [head_dim, S] layout; V in natural
    [S, head_dim] layout (via a PE transpose of VT)
  - scoresT[t, q] = KT_tile^T @ QT  -> exp on ACT (no max subtraction:
    scores are ~N(0,1) after folding 1/sqrt(D) into Wq, exp is safe)
  - softmax denominators via an all-ones stationary matmul (partition
    reduction on PE); the redundant 128 identical rows make the
    reciprocal + normalize plain full-tile DVE ops (no broadcasts)
  - attention output is accumulated transposed (outT[d, q]) so the
    output projection needs no transpose either; the host transposes
    the final [E, S] partial back to [S, E].
"""

import numpy as np

# problem shape (hardcoded per contract)
B, S, E = 2, 2048, 2048
H, G, D = 16, 4, 128
R = H // G          # heads per kv group = 4
KV = G * D          # 512
ST = S // 128       # 16 t-tiles
ET = E // 128       # 16 e-tiles
SC = S // 512       # 4 s-chunks
NPAIR = S // 1024   # 2 q-chunk pairs

_cache = {}


def _split_multi_waits(nc, maxw=1):
    """Walrus in this container accepts only one sync-wait per
    instruction; move extra waits onto preceding same-engine NoOps."""
    from concourse import mybir

    n_split = 0
    for fn in nc.m.functions:
        for bb in fn.blocks:
            out = []
            changed = False
            for inst in bb.instructions:
                si = inst.sync_info
                waits = list(si.on_wait or []) if si is not None else []
                if len(waits) > maxw:
                    changed = True
                    n_split += 1
                    head, tail = waits[:-maxw], waits[-maxw:]
                    for j in range(0, len(head), maxw):
                        nop = mybir.InstNoOp(
                            name=f"{inst.name}-wsplit{j}", ins=[], outs=[]
                        )
                        nop.engine = inst.engine
                        nop.sync_info = mybir.SyncInfo(
                            on_wait=head[j : j + maxw], on_update=[]
                        )
                        out.append(nop)
                    si.on_wait = tail
                out.append(inst)
            if changed:
                bb.instructions = out
    return n_split


def _build_program():
    import concourse.bass as bass
    import concourse.tile as tile
    from concourse import mybir
    from concourse.masks import make_identity

    F32R = mybir.dt.float32r
    F32 = mybir.dt.float32
    Exp = mybir.ActivationFunctionType.Exp
    Mult = mybir.AluOpType.mult

    nc = bass.Bass(target_bir_lowering=False)

    xT = nc.dram_tensor("xT", [E, S], F32R, kind="ExternalInput")
    wq = nc.dram_tensor("wq", [E, R * D], F32R, kind="ExternalInput")
    wk = nc.dram_tensor("wk", [E, D], F32R, kind="ExternalInput")
    wv = nc.dram_tensor("wv", [E, D], F32R, kind="ExternalInput")
    wo = nc.dram_tensor("wo", [R * D, E], F32R, kind="ExternalInput")
    bqv = nc.dram_tensor("bqv", [R * D], F32, kind="ExternalInput")
    bkv = nc.dram_tensor("bkv", [D], F32, kind="ExternalInput")
    bvv = nc.dram_tensor("bvv", [D], F32, kind="ExternalInput")
    otd = nc.dram_tensor("ot", [E, S], F32, kind="ExternalOutput")

    with tile.TileContext(nc) as tc:
        import contextlib

        with contextlib.ExitStack() as ctx:
            consts = ctx.enter_context(tc.tile_pool(name="consts", bufs=1))
            qkvt = ctx.enter_context(tc.tile_pool(name="qkvt", bufs=1))

            ident_f = consts.tile([128, 128], F32)
            make_identity(nc, ident_f)
            ident = consts.tile([128, 128], F32R)
            nc.vector.tensor_copy(ident, ident_f)
            ones_f = consts.tile([128, 128], F32)
            nc.gpsimd.memset(ones_f, 1.0)
            ones = consts.tile([128, 128], F32R)
            nc.vector.tensor_copy(ones, ones_f)
            bq_sb = consts.tile([128, R], F32)
            nc.sync.dma_start(bq_sb, bqv.rearrange("(o p) -> p o", p=128))
            bk_sb = consts.tile([128, 1], F32)
            nc.sync.dma_start(bk_sb, bkv.rearrange("(o p) -> p o", p=128))
            bv_sb = consts.tile([128, 1], F32)
            nc.sync.dma_start(bv_sb, bvv.rearrange("(o p) -> p o", p=128))

            QT = qkvt.tile([128, R, S], F32R)    # QT[d, h, s]
            KT = qkvt.tile([128, S], F32R)       # KT[d, t]
            V = qkvt.tile([128, ST, D], F32R)    # V[t%128, tt, d]

            # ---- phase 1: QKV^T projections + V transpose ----
            with tc.tile_pool(name="wts", bufs=1) as wpool, \
                 tc.tile_pool(name="xts", bufs=2) as xtpool, \
                 tc.tile_pool(name="vt", bufs=1) as vtpool, \
                 tc.tile_pool(name="ps1", bufs=3, space="PSUM") as ps1, \
                 tc.tile_pool(name="psv", bufs=2, space="PSUM") as psv:
                wq_sb = wpool.tile([128, ET, R * D], F32R)
                nc.sync.dma_start(wq_sb, wq.rearrange("(o p) m -> p o m", p=128))
                wk_sb = wpool.tile([128, ET, D], F32R)
                nc.sync.dma_start(wk_sb, wk.rearrange("(o p) m -> p o m", p=128))
                wv_sb = wpool.tile([128, ET, D], F32R)
                nc.sync.dma_start(wv_sb, wv.rearrange("(o p) m -> p o m", p=128))
                VT = vtpool.tile([128, S], F32R)

                for sc in range(SC):
                    xtile = xtpool.tile([128, ET, 512], F32R, tag="xt")
                    for e in range(ET):
                        nc.sync.dma_start(
                            xtile[:, e],
                            xT[e * 128 : (e + 1) * 128, sc * 512 : (sc + 1) * 512],
                        )
                    cs = slice(sc * 512, (sc + 1) * 512)
                    for ot in range(R + 2):
                        psum = ps1.tile([128, 512], F32, tag="p1")
                        for e in range(ET):
                            if ot < R:
                                lhsT = wq_sb[:, e, ot * 128 : (ot + 1) * 128]
                            elif ot == R:
                                lhsT = wk_sb[:, e]
                            else:
                                lhsT = wv_sb[:, e]
                            nc.tensor.matmul(
                                psum, lhsT, xtile[:, e],
                                start=(e == 0), stop=(e == ET - 1),
                            )
                        if ot < R:
                            nc.scalar.add(QT[:, ot, cs], psum, bq_sb[:, ot : ot + 1])
                        elif ot == R:
                            nc.scalar.add(KT[:, cs], psum, bk_sb[:, 0:1])
                        else:
                            nc.scalar.add(VT[:, cs], psum, bv_sb[:, 0:1])

                for tt in range(ST):
                    ps = psv.tile([128, 128], F32R, tag="pv")
                    nc.tensor.transpose(ps, VT[:, tt * 128 : (tt + 1) * 128], ident)
                    nc.vector.tensor_copy(V[:, tt], ps)

            # ---- phase 2: attention per head ----
            p23 = ctx.enter_context(tc.tile_pool(name="p23", bufs=1))
            outT = p23.tile([128, R, S], F32R)  # normalized attn outT[d, h, s]
            wo_sb = p23.tile([128, R, E], F32R)
            nc.sync.dma_start(wo_sb, wo.rearrange("(o p) m -> p o m", p=128))
            with tc.tile_pool(name="probs", bufs=3) as probs_pool, \
                 tc.tile_pool(name="recip", bufs=2) as rpool, \
                 tc.tile_pool(name="ps_s", bufs=2, space="PSUM") as ps_s, \
                 tc.tile_pool(name="ps_sum", bufs=1, space="PSUM") as ps_sum, \
                 tc.tile_pool(name="ps_av", bufs=1, space="PSUM") as ps_av:

                for h in range(R):
                    for pr in range(NPAIR):
                        q0 = pr * 1024
                        sums_ps = ps_sum.tile([128, 1024], F32, tag="sums")
                        out_ps = ps_av.tile([128, 1024], F32, tag="av")
                        for tt in range(ST):
                            pss = ps_s.tile([128, 1024], F32, tag="scores")
                            kslice = KT[:, tt * 128 : (tt + 1) * 128]
                            for hf in range(2):
                                nc.tensor.matmul(
                                    pss[:, hf * 512 : (hf + 1) * 512],
                                    kslice,
                                    QT[:, h, q0 + hf * 512 : q0 + (hf + 1) * 512],
                                    start=True, stop=True,
                                )
                            pt = probs_pool.tile([128, 1024], F32R, tag="probs")
                            nc.scalar.activation(pt, pss, Exp)
                            for hf in range(2):
                                hs = slice(hf * 512, (hf + 1) * 512)
                                nc.tensor.matmul(
                                    sums_ps[:, hs], ones, pt[:, hs],
                                    start=(tt == 0), stop=(tt == ST - 1),
                                )
                                nc.tensor.matmul(
                                    out_ps[:, hs], V[:, tt], pt[:, hs],
                                    start=(tt == 0), stop=(tt == ST - 1),
                                )
                        rc = rpool.tile([128, 1024], F32, tag="recip")
                        nc.vector.reciprocal(rc, sums_ps)
                        nc.vector.tensor_tensor(
                            outT[:, h, q0 : q0 + 1024], out_ps, rc, Mult
                        )

            # ---- phase 3: output projection (transposed) ----
            with tc.tile_pool(name="ostage", bufs=3) as ostage, \
                 tc.tile_pool(name="ps_o", bufs=4, space="PSUM") as ps_o:
                for et in range(ET):
                    for sc in range(SC):
                        ps = ps_o.tile([128, 512], F32, tag="po")
                        for h in range(R):
                            nc.tensor.matmul(
                                ps,
                                wo_sb[:, h, et * 128 : (et + 1) * 128],
                                outT[:, h, sc * 512 : (sc + 1) * 512],
                                start=(h == 0), stop=(h == R - 1),
                            )
                        st = ostage.tile([128, 512], F32, tag="ost")
                        nc.vector.tensor_copy(st, ps)
                        nc.sync.dma_start(
                            otd[et * 128 : (et + 1) * 128,
                                sc * 512 : (sc + 1) * 512],
                            st,
                        )

    _split_multi_waits(nc)
    return nc


def _prepare(x, Wq, bq, Wk, bk, Wv, bv, Wo, bo):
    """Host-side sharding: build per-core input maps."""
    x = np.asarray(x, dtype=np.float32)
    Wq = np.asarray(Wq, dtype=np.float32)
    bq = np.asarray(bq, dtype=np.float32)
    Wk = np.asarray(Wk, dtype=np.float32)
    bk = np.asarray(bk, dtype=np.float32)
    Wv = np.asarray(Wv, dtype=np.float32)
    bv = np.asarray(bv, dtype=np.float32)
    Wo = np.asarray(Wo, dtype=np.float32)

    isd = np.float32(1.0 / np.sqrt(D))
    xTs = [np.ascontiguousarray(x[b].T) for b in range(B)]
    in_maps = []
    for core in range(8):
        b, g = divmod(core, G)
        in_maps.append({
            "xT": xTs[b],
            "wq": np.ascontiguousarray(Wq[:, g * R * D : (g + 1) * R * D]) * isd,
            "wk": np.ascontiguousarray(Wk[:, g * D : (g + 1) * D]),
            "wv": np.ascontiguousarray(Wv[:, g * D : (g + 1) * D]),
            "wo": np.ascontiguousarray(Wo[g * R * D : (g + 1) * R * D, :]),
            "bqv": bq[g * R * D : (g + 1) * R * D] * isd,
            "bkv": bk[g * D : (g + 1) * D],
            "bvv": bv[g * D : (g + 1) * D],
        })
    return in_maps


def _gather(results, bo):
    bo = np.asarray(bo, dtype=np.float32)
    out = np.empty((B, S, E), dtype=np.float32)
    for b in range(B):
        acc = results[b * G]["ot"].copy()
        for g in range(1, G):
            acc += results[b * G + g]["ot"]
        out[b] = acc.T + bo
    return out


def kernel(x, Wq, bq, Wk, bk, Wv, bv, Wo, bo):
    from concourse.bass_utils import run_bass_kernel_spmd

    if "nc" not in _cache:
        _cache["nc"] = _build_program()
    nc = _cache["nc"]
    in_maps = _prepare(x, Wq, bq, Wk, bk, Wv, bv, Wo, bo)
    res = run_bass_kernel_spmd(nc, in_maps, core_ids=list(range(8)))
    return _gather(res.results, bo)



# revision 8
# speedup vs baseline: 1.1866x; 1.1866x over previous
"""GQA attention kernel for 8 Trainium2 NeuronCores.

Sharding: core = (batch b, kv_group g), b in {0,1}, g in {0..3}.
Each core computes the 4 heads of one KV group for one batch and the
partial output projection for those heads; the host sums the 4 group
partials per batch.  Zero duplicated compute across cores.

v2 changes vs baseline (516us):
  - P1 (QKV proj) restructured e-outer with 6 concurrent PSUM
    accumulation groups + quarter-chunk weight/x DMAs: compute starts
    ~4us in instead of ~34us, and PE never waits on DMA.
  - P2 (attention) software-pipelined with lookahead-2 scores in a
    3-deep PSUM rotation so PE never stalls on the ACT exp chain
    (was ~740ns stall per tile + 5.5us per block boundary).
  - softmax denominators moved off TensorE: probs are written bf16,
    DVE accumulates prob tiles (2x fast mode for 2-byte dtypes), one
    final ones-matmul per (head, q-half) block reduces the 128-row
    accumulator.  Saves ~65us of PE sums-matmul time.
  - normalization uses reciprocal_approx_fast (5x faster than DVE
    reciprocal) and is deferred off the critical path; the AV PSUM
    accumulator is drained by split DVE/Pool copies so the next
    block's AV matmuls never wait on the normalize chain.
  - numerics: scores/Q/K/weights stay fp32r; only probs/V/acc are
    bf16 (validated 2.1e-3 max rel err vs 2e-2 budget).
"""

import numpy as np

# problem shape (hardcoded per contract)
B, S, E = 2, 2048, 2048
H, G, D = 16, 4, 128
R = H // G          # heads per kv group = 4
KV = G * D          # 512
ST = S // 128       # 16 t-tiles
ET = E // 128       # 16 e-tiles
SC = S // 512       # 4 s-chunks
NPAIR = S // 1024   # 2 q-chunk pairs
NBLK = R * NPAIR    # 8 attention blocks per core
LOOK = 2            # scores lookahead (PSUM rotation depth - 1)

_cache = {}


def _split_multi_waits(nc, maxw=1):
    """Walrus in this container accepts only one sync-wait per
    instruction; move extra waits onto preceding same-engine NoOps."""
    from concourse import mybir

    n_split = 0
    for fn in nc.m.functions:
        for bb in fn.blocks:
            out = []
            changed = False
            for inst in bb.instructions:
                si = inst.sync_info
                waits = list(si.on_wait or []) if si is not None else []
                if len(waits) > maxw:
                    changed = True
                    n_split += 1
                    head, tail = waits[:-maxw], waits[-maxw:]
                    for j in range(0, len(head), maxw):
                        nop = mybir.InstNoOp(
                            name=f"{inst.name}-wsplit{j}", ins=[], outs=[]
                        )
                        nop.engine = inst.engine
                        nop.sync_info = mybir.SyncInfo(
                            on_wait=head[j : j + maxw], on_update=[]
                        )
                        out.append(nop)
                    si.on_wait = tail
                out.append(inst)
            if changed:
                bb.instructions = out
    return n_split


def _build_program():
    import concourse.bass as bass
    import concourse.tile as tile
    from concourse import mybir
    from concourse.masks import make_identity

    F32R = mybir.dt.float32r
    F32 = mybir.dt.float32
    BF16 = mybir.dt.bfloat16
    Exp = mybir.ActivationFunctionType.Exp
    Mult = mybir.AluOpType.mult
    Add = mybir.AluOpType.add

    nc = bass.Bass(target_bir_lowering=False)

    xT = nc.dram_tensor("xT", [E, S], F32R, kind="ExternalInput")
    wq = nc.dram_tensor("wq", [E, R * D], F32R, kind="ExternalInput")
    wk = nc.dram_tensor("wk", [E, D], F32R, kind="ExternalInput")
    wv = nc.dram_tensor("wv", [E, D], F32R, kind="ExternalInput")
    wo = nc.dram_tensor("wo", [R * D, E], F32R, kind="ExternalInput")
    bqv = nc.dram_tensor("bqv", [R * D], F32, kind="ExternalInput")
    bkv = nc.dram_tensor("bkv", [D], F32, kind="ExternalInput")
    bvv = nc.dram_tensor("bvv", [D], F32, kind="ExternalInput")
    otd = nc.dram_tensor("ot", [E, S], F32, kind="ExternalOutput")

    xTr = xT.rearrange("(o p) m -> p o m", p=128)
    wqr = wq.rearrange("(o p) m -> p o m", p=128)
    wkr = wk.rearrange("(o p) m -> p o m", p=128)
    wvr = wv.rearrange("(o p) m -> p o m", p=128)
    wor = wo.rearrange("(o p) m -> p o m", p=128)

    with tile.TileContext(nc) as tc:
        import contextlib

        with contextlib.ExitStack() as ctx:
            consts = ctx.enter_context(tc.tile_pool(name="consts", bufs=1))
            qkvt = ctx.enter_context(tc.tile_pool(name="qkvt", bufs=1))

            ident_f = consts.tile([128, 128], F32)
            make_identity(nc, ident_f)
            ident = consts.tile([128, 128], F32R)
            nc.vector.tensor_copy(ident, ident_f)
            ones_bf = consts.tile([128, 128], BF16)
            nc.gpsimd.memset(ones_bf, 1.0)
            bq_sb = consts.tile([128, R], F32)
            nc.sync.dma_start(bq_sb, bqv.rearrange("(o p) -> p o", p=128))
            bk_sb = consts.tile([128, 1], F32)
            nc.sync.dma_start(bk_sb, bkv.rearrange("(o p) -> p o", p=128))
            bv_sb = consts.tile([128, 1], F32)
            nc.sync.dma_start(bv_sb, bvv.rearrange("(o p) -> p o", p=128))

            QT = qkvt.tile([128, R, S], F32R)    # QT[d, h, s]
            KT = qkvt.tile([128, S], F32R)       # KT[d, t]
            V = qkvt.tile([128, ST, D], BF16)    # V[t%128, tt, d]

            # ---- phase 1: QKV^T projections ----
            with tc.tile_pool(name="vt", bufs=1) as vtpool:
                VT = vtpool.tile([128, S], F32R)
                with tc.tile_pool(name="wts", bufs=1) as wpool, \
                     tc.tile_pool(name="xts", bufs=3) as xtpool, \
                     tc.tile_pool(name="ps1", bufs=8, space="PSUM") as ps1:
                    wq_sb = wpool.tile([128, ET, R * D], F32R)
                    wk_sb = wpool.tile([128, ET, D], F32R)
                    wv_sb = wpool.tile([128, ET, D], F32R)
                    # quarter-chunk weight DMAs in e-order so e=0..3 of all
                    # three weights land before e=4.. of any
                    for q in range(4):
                        sl = slice(q * 4, q * 4 + 4)
                        nc.sync.dma_start(wq_sb[:, sl], wqr[:, sl])
                        nc.sync.dma_start(wk_sb[:, sl], wkr[:, sl])
                        nc.sync.dma_start(wv_sb[:, sl], wvr[:, sl])

                    for sc in range(SC):
                        cs = slice(sc * 512, (sc + 1) * 512)
                        po = [ps1.tile([128, 512], F32, tag="p1", name="po")
                              for _ in range(R + 2)]
                        for eq in range(4):
                            xq = xtpool.tile([128, 4, 512], F32R, tag="xt")
                            nc.gpsimd.dma_start(
                                xq, xTr[:, eq * 4 : eq * 4 + 4, cs]
                            )
                            for i in range(4):
                                e = eq * 4 + i
                                for ot in range(R + 2):
                                    if ot < R:
                                        lhsT = wq_sb[:, e, ot * 128 : (ot + 1) * 128]
                                    elif ot == R:
                                        lhsT = wk_sb[:, e]
                                    else:
                                        lhsT = wv_sb[:, e]
                                    nc.tensor.matmul(
                                        po[ot], lhsT, xq[:, i],
                                        start=(e == 0), stop=(e == ET - 1),
                                    )
                        for ot in range(R):
                            nc.scalar.add(QT[:, ot, cs], po[ot], bq_sb[:, ot : ot + 1])
                        nc.scalar.add(KT[:, cs], po[R], bk_sb[:, 0:1])
                        nc.scalar.add(VT[:, cs], po[R + 1], bv_sb[:, 0:1])

                # V transpose (PE) after projection PSUM pool is closed
                with tc.tile_pool(name="psv", bufs=2, space="PSUM") as psv:
                    for tt in range(ST):
                        ps = psv.tile([128, 128], F32R, tag="pv")
                        nc.tensor.transpose(
                            ps, VT[:, tt * 128 : (tt + 1) * 128], ident
                        )
                        nc.vector.tensor_copy(V[:, tt], ps)

            # ---- phase 2: attention, software-pipelined ----
            p23 = ctx.enter_context(tc.tile_pool(name="p23", bufs=1))
            outT = p23.tile([128, R, S], F32R)  # normalized attn outT[d, h, s]
            wo_sb = p23.tile([128, R, E], F32R)
            for q in range(4):
                nc.sync.dma_start(wo_sb[:, q], wor[:, q])

            with tc.tile_pool(name="ps_mix", bufs=3, space="PSUM") as ps_mix, \
                 tc.tile_pool(name="ps_av", bufs=1, space="PSUM") as ps_av, \
                 tc.tile_pool(name="probs", bufs=3) as probs_pool, \
                 tc.tile_pool(name="accp", bufs=2) as accp, \
                 tc.tile_pool(name="avsb", bufs=2) as avsb, \
                 tc.tile_pool(name="rcp", bufs=2) as rcp:

                pss_t = {}   # j -> scores psum tile
                acc_t = {}   # blk -> (dve_acc, pool_acc) bf16 accumulators
                av_t = {}    # blk -> AV psum tile
                avsb_t = {}  # blk -> AV sbuf drain tile

                def finalize(b):
                    """Deferred per-block tail: reduce the prob accumulators,
                    reciprocal, and normalize into outT.  Runs 3+ tiles into
                    the next block so nothing on PE ever waits for it."""
                    h, pr = b // NPAIR, b % NPAIR
                    q0 = pr * 1024
                    acc_d, acc_p = acc_t.pop(b)
                    sums = ps_mix.tile([128, 1024], F32, tag="s")
                    for hf in range(2):
                        hs = slice(hf * 512, (hf + 1) * 512)
                        nc.tensor.matmul(
                            sums[:, hs], ones_bf, acc_d[:, hs],
                            start=True, stop=False,
                        )
                        nc.tensor.matmul(
                            sums[:, hs], ones_bf, acc_p[:, hs],
                            start=False, stop=True,
                        )
                    rc = rcp.tile([128, 1024], F32, tag="rc")
                    nc.vector.reciprocal(rc, sums)
                    nc.vector.tensor_tensor(
                        outT[:, h, q0 : q0 + 1024], avsb_t.pop(b), rc, Mult
                    )

                for j in range(16 * NBLK + LOOK + 5):
                    if j >= 20 and (j - 20) % 16 == 0 and (j - 20) // 16 < NBLK:
                        finalize((j - 20) // 16)
                    if j < 16 * NBLK:
                        b, tt = j // ST, j % ST
                        h, pr = b // NPAIR, b % NPAIR
                        q0 = pr * 1024
                        pss = ps_mix.tile([128, 1024], F32, tag="s")
                        kslice = KT[:, tt * 128 : (tt + 1) * 128]
                        for hf in range(2):
                            nc.tensor.matmul(
                                pss[:, hf * 512 : (hf + 1) * 512],
                                kslice,
                                QT[:, h, q0 + hf * 512 : q0 + (hf + 1) * 512],
                                start=True, stop=True,
                            )
                        pss_t[j] = pss
                    jj = j - LOOK
                    if 0 <= jj < 16 * NBLK:
                        b, tt = jj // ST, jj % ST
                        h, pr = b // NPAIR, b % NPAIR
                        q0 = pr * 1024
                        pt = probs_pool.tile([128, 1024], BF16, tag="pt")
                        nc.scalar.activation(pt, pss_t.pop(jj), Exp)
                        # prob-tile accumulation for the softmax denominator:
                        # even tiles on DVE, odd tiles on Pool, two
                        # independent accumulators so neither chain waits on
                        # the other engine
                        if tt == 0:
                            acc_d = accp.tile([128, 1024], BF16, tag="accd")
                            acc_p = accp.tile([128, 1024], BF16, tag="accp")
                            acc_t[b] = (acc_d, acc_p)
                            nc.vector.tensor_copy(acc_d, pt)
                            av_t[b] = ps_av.tile(
                                [128, 1024], F32, tag="av", name="avp"
                            )
                        elif tt == 1:
                            nc.gpsimd.tensor_copy(acc_t[b][1], pt)
                        elif tt % 2 == 0:
                            acc_d = acc_t[b][0]
                            nc.vector.tensor_tensor(acc_d, acc_d, pt, Add)
                        else:
                            acc_p = acc_t[b][1]
                            nc.gpsimd.tensor_tensor(acc_p, acc_p, pt, Add)
                        av = av_t[b]
                        for hf in range(2):
                            hs = slice(hf * 512, (hf + 1) * 512)
                            nc.tensor.matmul(
                                av[:, hs], V[:, tt], pt[:, hs],
                                start=(tt == 0), stop=(tt == ST - 1),
                            )
                        if tt == ST - 1:
                            # drain AV psum immediately so the next block's
                            # AV start never waits on the normalize chain
                            asb = avsb.tile([128, 1024], F32, tag="avsb")
                            avsb_t[b] = asb
                            nc.vector.tensor_copy(asb, av_t.pop(b))

            # ---- phase 3: output projection (transposed) ----
            with tc.tile_pool(name="ostage", bufs=3) as ostage, \
                 tc.tile_pool(name="ps_o", bufs=4, space="PSUM") as ps_o:
                for et in range(ET):
                    for sc in range(SC):
                        ps = ps_o.tile([128, 512], F32, tag="po")
                        for h in range(R):
                            nc.tensor.matmul(
                                ps,
                                wo_sb[:, h, et * 128 : (et + 1) * 128],
                                outT[:, h, sc * 512 : (sc + 1) * 512],
                                start=(h == 0), stop=(h == R - 1),
                            )
                        st = ostage.tile([128, 512], F32, tag="ost")
                        nc.vector.tensor_copy(st, ps)
                        nc.sync.dma_start(
                            otd[et * 128 : (et + 1) * 128,
                                sc * 512 : (sc + 1) * 512],
                            st,
                        )

    _split_multi_waits(nc)
    return nc


def _prepare(x, Wq, bq, Wk, bk, Wv, bv, Wo, bo):
    """Host-side sharding: build per-core input maps."""
    x = np.asarray(x, dtype=np.float32)
    Wq = np.asarray(Wq, dtype=np.float32)
    bq = np.asarray(bq, dtype=np.float32)
    Wk = np.asarray(Wk, dtype=np.float32)
    bk = np.asarray(bk, dtype=np.float32)
    Wv = np.asarray(Wv, dtype=np.float32)
    bv = np.asarray(bv, dtype=np.float32)
    Wo = np.asarray(Wo, dtype=np.float32)

    isd = np.float32(1.0 / np.sqrt(D))
    xTs = [np.ascontiguousarray(x[b].T) for b in range(B)]
    in_maps = []
    for core in range(8):
        b, g = divmod(core, G)
        in_maps.append({
            "xT": xTs[b],
            "wq": np.ascontiguousarray(Wq[:, g * R * D : (g + 1) * R * D]) * isd,
            "wk": np.ascontiguousarray(Wk[:, g * D : (g + 1) * D]),
            "wv": np.ascontiguousarray(Wv[:, g * D : (g + 1) * D]),
            "wo": np.ascontiguousarray(Wo[g * R * D : (g + 1) * R * D, :]),
            "bqv": bq[g * R * D : (g + 1) * R * D] * isd,
            "bkv": bk[g * D : (g + 1) * D],
            "bvv": bv[g * D : (g + 1) * D],
        })
    return in_maps


def _gather(results, bo):
    bo = np.asarray(bo, dtype=np.float32)
    out = np.empty((B, S, E), dtype=np.float32)
    for b in range(B):
        acc = results[b * G]["ot"].copy()
        for g in range(1, G):
            acc += results[b * G + g]["ot"]
        out[b] = acc.T + bo
    return out


def kernel(x, Wq, bq, Wk, bk, Wv, bv, Wo, bo):
    from concourse.bass_utils import run_bass_kernel_spmd

    if "nc" not in _cache:
        _cache["nc"] = _build_program()
    nc = _cache["nc"]
    in_maps = _prepare(x, Wq, bq, Wk, bk, Wv, bv, Wo, bo)
    res = run_bass_kernel_spmd(nc, in_maps, core_ids=list(range(8)))
    return _gather(res.results, bo)


# revision 15
# speedup vs baseline: 1.3057x; 1.1004x over previous
"""GQA attention kernel for 8 Trainium2 NeuronCores.

Sharding: core = (batch b, kv_group g), b in {0,1}, g in {0..3}.
Each core computes the 4 heads of one KV group for one batch and the
partial output projection for those heads; the host sums the 4 group
partials per batch.  Zero duplicated compute across cores.

v2 changes vs baseline (516us):
  - P1 (QKV proj) restructured e-outer with 6 concurrent PSUM
    accumulation groups + quarter-chunk weight/x DMAs: compute starts
    ~4us in instead of ~34us, and PE never waits on DMA.
  - P2 (attention) software-pipelined with lookahead-2 scores in a
    3-deep PSUM rotation so PE never stalls on the ACT exp chain
    (was ~740ns stall per tile + 5.5us per block boundary).
  - softmax denominators moved off TensorE: probs are written bf16,
    DVE accumulates prob tiles (2x fast mode for 2-byte dtypes), one
    final ones-matmul per (head, q-half) block reduces the 128-row
    accumulator.  Saves ~65us of PE sums-matmul time.
  - normalization uses reciprocal_approx_fast (5x faster than DVE
    reciprocal) and is deferred off the critical path; the AV PSUM
    accumulator is drained by split DVE/Pool copies so the next
    block's AV matmuls never wait on the normalize chain.
  - numerics: scores/Q/K/weights stay fp32r; only probs/V/acc are
    bf16 (validated 2.1e-3 max rel err vs 2e-2 budget).
"""

import numpy as np

# problem shape (hardcoded per contract)
B, S, E = 2, 2048, 2048
H, G, D = 16, 4, 128
R = H // G          # heads per kv group = 4
KV = G * D          # 512
ST = S // 128       # 16 t-tiles
ET = E // 128       # 16 e-tiles
SC = S // 512       # 4 s-chunks
NPAIR = S // 1024   # 2 q-chunk pairs
NBLK = R * NPAIR    # 8 attention blocks per core
LOOK = 2            # scores lookahead (PSUM rotation depth - 1)

_cache = {}


def _split_multi_waits(nc, maxw=1):
    """Walrus in this container accepts only one sync-wait per
    instruction; move extra waits onto preceding same-engine NoOps."""
    from concourse import mybir

    n_split = 0
    for fn in nc.m.functions:
        for bb in fn.blocks:
            out = []
            changed = False
            for inst in bb.instructions:
                si = inst.sync_info
                waits = list(si.on_wait or []) if si is not None else []
                if len(waits) > maxw:
                    changed = True
                    n_split += 1
                    head, tail = waits[:-maxw], waits[-maxw:]
                    for j in range(0, len(head), maxw):
                        nop = mybir.InstNoOp(
                            name=f"{inst.name}-wsplit{j}", ins=[], outs=[]
                        )
                        nop.engine = inst.engine
                        nop.sync_info = mybir.SyncInfo(
                            on_wait=head[j : j + maxw], on_update=[]
                        )
                        out.append(nop)
                    si.on_wait = tail
                out.append(inst)
            if changed:
                bb.instructions = out
    return n_split


def _build_program():
    import concourse.bass as bass
    import concourse.tile as tile
    from concourse import mybir
    from concourse.masks import make_identity

    F32R = mybir.dt.float32r
    F32 = mybir.dt.float32
    BF16 = mybir.dt.bfloat16
    Exp = mybir.ActivationFunctionType.Exp
    Mult = mybir.AluOpType.mult
    Add = mybir.AluOpType.add

    nc = bass.Bass(target_bir_lowering=False)

    xT = nc.dram_tensor("xT", [E, S], F32R, kind="ExternalInput")
    wq = nc.dram_tensor("wq", [E, R * D], F32R, kind="ExternalInput")
    wk = nc.dram_tensor("wk", [E, D], F32R, kind="ExternalInput")
    wv = nc.dram_tensor("wv", [E, D], F32R, kind="ExternalInput")
    wo = nc.dram_tensor("wo", [R * D, E], F32R, kind="ExternalInput")
    bqv = nc.dram_tensor("bqv", [R * D], F32, kind="ExternalInput")
    bkv = nc.dram_tensor("bkv", [D], F32, kind="ExternalInput")
    bvv = nc.dram_tensor("bvv", [D], F32, kind="ExternalInput")
    otd = nc.dram_tensor("ot", [E, S], F32, kind="ExternalOutput")

    xTr = xT.rearrange("(o p) m -> p o m", p=128)
    wqr = wq.rearrange("(o p) m -> p o m", p=128)
    wkr = wk.rearrange("(o p) m -> p o m", p=128)
    wvr = wv.rearrange("(o p) m -> p o m", p=128)
    wor = wo.rearrange("(o p) m -> p o m", p=128)

    with tile.TileContext(nc) as tc:
        import contextlib

        with contextlib.ExitStack() as ctx:
            consts = ctx.enter_context(tc.tile_pool(name="consts", bufs=1))
            qkvt = ctx.enter_context(tc.tile_pool(name="qkvt", bufs=1))

            ident_f = consts.tile([128, 128], F32)
            make_identity(nc, ident_f)
            ident = consts.tile([128, 128], F32R)
            nc.vector.tensor_copy(ident, ident_f)
            ones_bf = consts.tile([128, 128], BF16)
            nc.gpsimd.memset(ones_bf, 1.0)
            bq_sb = consts.tile([128, R], F32)
            nc.sync.dma_start(bq_sb, bqv.rearrange("(o p) -> p o", p=128))
            bk_sb = consts.tile([128, 1], F32)
            nc.sync.dma_start(bk_sb, bkv.rearrange("(o p) -> p o", p=128))
            bv_sb = consts.tile([128, 1], F32)
            nc.sync.dma_start(bv_sb, bvv.rearrange("(o p) -> p o", p=128))

            QT = qkvt.tile([128, R, S], F32R)    # QT[d, h, s]
            KT = qkvt.tile([128, S], F32R)       # KT[d, t]
            V = qkvt.tile([128, ST, D], BF16)    # V[t%128, tt, d]

            # ---- phase 1: QKV^T projections ----
            with tc.tile_pool(name="vt", bufs=1) as vtpool:
                VT = vtpool.tile([128, S], F32R)
                with tc.tile_pool(name="wts", bufs=1) as wpool, \
                     tc.tile_pool(name="xts", bufs=3) as xtpool, \
                     tc.tile_pool(name="ps1", bufs=8, space="PSUM") as ps1:
                    wq_sb = wpool.tile([128, ET, R * D], F32R)
                    wk_sb = wpool.tile([128, ET, D], F32R)
                    wv_sb = wpool.tile([128, ET, D], F32R)
                    # e-granular DMAs for the first quarter (so the first
                    # matmuls unblock asap), bulk quarters after; wk/wv on
                    # the scalar queue to parallelize descriptor generation
                    for e in range(4):
                        sl = slice(e, e + 1)
                        nc.sync.dma_start(wq_sb[:, sl], wqr[:, sl])
                        nc.scalar.dma_start(wk_sb[:, sl], wkr[:, sl])
                        nc.scalar.dma_start(wv_sb[:, sl], wvr[:, sl])
                    for q in range(1, 4):
                        sl = slice(q * 4, q * 4 + 4)
                        nc.sync.dma_start(wq_sb[:, sl], wqr[:, sl])
                        nc.scalar.dma_start(wk_sb[:, sl], wkr[:, sl])
                        nc.scalar.dma_start(wv_sb[:, sl], wvr[:, sl])

                    for sc in range(SC):
                        cs = slice(sc * 512, (sc + 1) * 512)
                        po = [ps1.tile([128, 512], F32, tag="p1", name="po")
                              for _ in range(R + 2)]
                        for eq in range(4):
                            xq = xtpool.tile([128, 4, 512], F32R, tag="xt")
                            if sc == 0 and eq == 0:
                                # e-granular so the first matmul starts early
                                for i in range(4):
                                    nc.gpsimd.dma_start(
                                        xq[:, i : i + 1],
                                        xTr[:, i : i + 1, cs],
                                    )
                            else:
                                nc.gpsimd.dma_start(
                                    xq, xTr[:, eq * 4 : eq * 4 + 4, cs]
                                )
                            for i in range(4):
                                e = eq * 4 + i
                                for ot in range(R + 2):
                                    if ot < R:
                                        lhsT = wq_sb[:, e, ot * 128 : (ot + 1) * 128]
                                    elif ot == R:
                                        lhsT = wk_sb[:, e]
                                    else:
                                        lhsT = wv_sb[:, e]
                                    nc.tensor.matmul(
                                        po[ot], lhsT, xq[:, i],
                                        start=(e == 0), stop=(e == ET - 1),
                                    )
                        for ot in range(R):
                            nc.scalar.add(QT[:, ot, cs], po[ot], bq_sb[:, ot : ot + 1])
                        nc.scalar.add(KT[:, cs], po[R], bk_sb[:, 0:1])
                        nc.scalar.add(VT[:, cs], po[R + 1], bv_sb[:, 0:1])

                # V transpose (PE) after projection PSUM pool is closed
                with tc.tile_pool(name="psv", bufs=2, space="PSUM") as psv:
                    for tt in range(ST):
                        ps = psv.tile([128, 128], F32R, tag="pv")
                        nc.tensor.transpose(
                            ps, VT[:, tt * 128 : (tt + 1) * 128], ident
                        )
                        nc.vector.tensor_copy(V[:, tt], ps)

            # ---- phase 2: attention, software-pipelined ----
            p23 = ctx.enter_context(tc.tile_pool(name="p23", bufs=1))
            outT = p23.tile([128, R, S], F32R)  # normalized attn outT[d, h, s]
            wo_sb = p23.tile([128, R, E], F32R)
            for q in range(4):
                nc.sync.dma_start(wo_sb[:, q], wor[:, q])

            with tc.tile_pool(name="ps_mix", bufs=3, space="PSUM") as ps_mix, \
                 tc.tile_pool(name="ps_av", bufs=1, space="PSUM") as ps_av, \
                 tc.tile_pool(name="probs", bufs=8) as probs_pool, \
                 tc.tile_pool(name="accp", bufs=2) as accp, \
                 tc.tile_pool(name="avsb", bufs=2) as avsb, \
                 tc.tile_pool(name="rcp", bufs=2) as rcp:

                pss_t = {}   # j -> scores psum tile
                acc_t = {}   # blk -> (dve_acc, pool_acc) bf16 accumulators
                av_t = {}    # blk -> AV psum tile
                avsb_t = {}  # blk -> AV sbuf drain tile
                pt_tail = {}  # blk -> [pt(14), pt(15)] summed directly on PE

                def finalize(b):
                    """Deferred per-block tail: reduce the prob accumulators
                    (+ the two tail prob tiles, summed directly on PE to
                    offload DVE/Pool), reciprocal, and normalize into outT.
                    Runs 3+ tiles into the next block so nothing on PE ever
                    waits for it."""
                    h, pr = b // NPAIR, b % NPAIR
                    q0 = pr * 1024
                    acc_d, acc_p = acc_t.pop(b)
                    srcs = [acc_d, acc_p] + pt_tail.pop(b)
                    sums = ps_mix.tile([128, 1024], F32, tag="s")
                    for hf in range(2):
                        hs = slice(hf * 512, (hf + 1) * 512)
                        for si, src in enumerate(srcs):
                            nc.tensor.matmul(
                                sums[:, hs], ones_bf, src[:, hs],
                                start=(si == 0), stop=(si == len(srcs) - 1),
                            )
                    rc = rcp.tile([128, 1024], F32, tag="rc")
                    nc.vector.reciprocal(rc, sums)
                    nc.vector.tensor_tensor(
                        outT[:, h, q0 : q0 + 1024], avsb_t.pop(b), rc, Mult
                    )

                for j in range(16 * NBLK + LOOK + 5):
                    if j >= 20 and (j - 20) % 16 == 0 and (j - 20) // 16 < NBLK:
                        finalize((j - 20) // 16)
                    if j < 16 * NBLK:
                        b, tt = j // ST, j % ST
                        h, pr = b // NPAIR, b % NPAIR
                        q0 = pr * 1024
                        pss = ps_mix.tile([128, 1024], F32, tag="s")
                        kslice = KT[:, tt * 128 : (tt + 1) * 128]
                        for hf in range(2):
                            nc.tensor.matmul(
                                pss[:, hf * 512 : (hf + 1) * 512],
                                kslice,
                                QT[:, h, q0 + hf * 512 : q0 + (hf + 1) * 512],
                                start=True, stop=True,
                            )
                        pss_t[j] = pss
                    jj = j - LOOK
                    if 0 <= jj < 16 * NBLK:
                        b, tt = jj // ST, jj % ST
                        h, pr = b // NPAIR, b % NPAIR
                        q0 = pr * 1024
                        pt = probs_pool.tile([128, 1024], BF16, tag="pt")
                        nc.scalar.activation(pt, pss_t.pop(jj), Exp)
                        # prob-tile accumulation for the softmax denominator:
                        # even tiles on DVE, odd tiles on Pool (independent
                        # accumulators so neither chain waits on the other
                        # engine); the last two tiles go straight to the PE
                        # sums-matmul in finalize()
                        if tt == 0:
                            acc_d = accp.tile([128, 1024], BF16, tag="accd")
                            acc_p = accp.tile([128, 1024], BF16, tag="accp")
                            acc_t[b] = (acc_d, acc_p)
                            pt_tail[b] = []
                            nc.vector.tensor_copy(acc_d, pt)
                            nc.gpsimd.memset(acc_p, 0.0)
                            av_t[b] = ps_av.tile(
                                [128, 1024], F32, tag="av", name="avp"
                            )
                        elif tt >= ST - 2:
                            pt_tail[b].append(pt)
                        elif tt % 2 == 0:
                            acc_d = acc_t[b][0]
                            nc.vector.tensor_tensor(acc_d, acc_d, pt, Add)
                        else:
                            acc_p = acc_t[b][1]
                            nc.gpsimd.tensor_tensor(acc_p, acc_p, pt, Add)
                        av = av_t[b]
                        for hf in range(2):
                            hs = slice(hf * 512, (hf + 1) * 512)
                            nc.tensor.matmul(
                                av[:, hs], V[:, tt], pt[:, hs],
                                start=(tt == 0), stop=(tt == ST - 1),
                            )
                        if tt == ST - 1:
                            # drain AV psum immediately so the next block's
                            # AV start never waits on the normalize chain
                            asb = avsb.tile([128, 1024], F32, tag="avsb")
                            avsb_t[b] = asb
                            nc.vector.tensor_copy(asb, av_t.pop(b))

            # ---- phase 3: output projection (transposed) ----
            with tc.tile_pool(name="ostage", bufs=4) as ostage, \
                 tc.tile_pool(name="ps_o", bufs=6, space="PSUM") as ps_o:
                # sc outer: the first tiles only need the pr=0 (even) blocks,
                # so P3 never waits on the last blocks' deferred normalize
                for sc in range(SC):
                    for et in range(ET):
                        ps = ps_o.tile([128, 512], F32, tag="po")
                        for h in range(R):
                            nc.tensor.matmul(
                                ps,
                                wo_sb[:, h, et * 128 : (et + 1) * 128],
                                outT[:, h, sc * 512 : (sc + 1) * 512],
                                start=(h == 0), stop=(h == R - 1),
                            )
                        st = ostage.tile([128, 512], F32, tag="ost")
                        nc.vector.tensor_copy(st, ps)
                        nc.sync.dma_start(
                            otd[et * 128 : (et + 1) * 128,
                                sc * 512 : (sc + 1) * 512],
                            st,
                        )

    _split_multi_waits(nc)
    return nc


def _prepare(x, Wq, bq, Wk, bk, Wv, bv, Wo, bo):
    """Host-side sharding: build per-core input maps."""
    x = np.asarray(x, dtype=np.float32)
    Wq = np.asarray(Wq, dtype=np.float32)
    bq = np.asarray(bq, dtype=np.float32)
    Wk = np.asarray(Wk, dtype=np.float32)
    bk = np.asarray(bk, dtype=np.float32)
    Wv = np.asarray(Wv, dtype=np.float32)
    bv = np.asarray(bv, dtype=np.float32)
    Wo = np.asarray(Wo, dtype=np.float32)

    isd = np.float32(1.0 / np.sqrt(D))
    xTs = [np.ascontiguousarray(x[b].T) for b in range(B)]
    in_maps = []
    for core in range(8):
        b, g = divmod(core, G)
        in_maps.append({
            "xT": xTs[b],
            "wq": np.ascontiguousarray(Wq[:, g * R * D : (g + 1) * R * D]) * isd,
            "wk": np.ascontiguousarray(Wk[:, g * D : (g + 1) * D]),
            "wv": np.ascontiguousarray(Wv[:, g * D : (g + 1) * D]),
            "wo": np.ascontiguousarray(Wo[g * R * D : (g + 1) * R * D, :]),
            "bqv": bq[g * R * D : (g + 1) * R * D] * isd,
            "bkv": bk[g * D : (g + 1) * D],
            "bvv": bv[g * D : (g + 1) * D],
        })
    return in_maps


def _gather(results, bo):
    bo = np.asarray(bo, dtype=np.float32)
    out = np.empty((B, S, E), dtype=np.float32)
    for b in range(B):
        acc = results[b * G]["ot"].copy()
        for g in range(1, G):
            acc += results[b * G + g]["ot"]
        out[b] = acc.T + bo
    return out


def kernel(x, Wq, bq, Wk, bk, Wv, bv, Wo, bo):
    from concourse.bass_utils import run_bass_kernel_spmd

    if "nc" not in _cache:
        _cache["nc"] = _build_program()
    nc = _cache["nc"]
    in_maps = _prepare(x, Wq, bq, Wk, bk, Wv, bv, Wo, bo)
    res = run_bass_kernel_spmd(nc, in_maps, core_ids=list(range(8)))
    return _gather(res.results, bo)


# revision 16
# speedup vs baseline: 1.3169x; 1.0085x over previous
"""GQA attention kernel for 8 Trainium2 NeuronCores.

Sharding: core = (batch b, kv_group g), b in {0,1}, g in {0..3}.
Each core computes the 4 heads of one KV group for one batch and the
partial output projection for those heads; the host sums the 4 group
partials per batch.  Zero duplicated compute across cores.

v4 design (baseline was 516us):
  - P1 (QKV proj) e-outer with 6 concurrent PSUM accumulation groups;
    e-granular first-quarter DMAs spread across the SP/ACT/Pool DGE
    queues so the first matmul starts ~12us in and PE never waits on
    DMA; V transposed through a side PSUM bank inside the loop.
  - P2 (attention) software-pipelined with lookahead-2 scores in a
    3-deep PSUM rotation so PE never stalls on the ACT exp chain.
  - softmax denominators: probs written bf16; accumulation split
    across DVE (5 tiles + init copy), Pool (7 tiles), and PE
    (3 tail tiles via the final ones-matmul), sized from measured
    per-op costs so every engine stays under PE's per-block time.
  - per-block normalize (reciprocal + multiply) deferred 4 tiles into
    the next block, AV PSUM drained immediately by DVE, so no PE
    instruction ever waits on the normalize chain; the last block's
    reduction uses the AV PSUM pool so phase 3's PSUM pool opens
    without waiting on it.
  - numerics: scores/Q/K/weights stay fp32r; only probs/V/acc are
    bf16 (validated 2.1e-3 max rel err vs 2e-2 budget).
"""

import numpy as np

# problem shape (hardcoded per contract)
B, S, E = 2, 2048, 2048
H, G, D = 16, 4, 128
R = H // G          # heads per kv group = 4
KV = G * D          # 512
ST = S // 128       # 16 t-tiles
ET = E // 128       # 16 e-tiles
SC = S // 512       # 4 s-chunks
NPAIR = S // 1024   # 2 q-chunk pairs
NBLK = R * NPAIR    # 8 attention blocks per core
LOOK = 2            # scores lookahead (PSUM rotation depth - 1)

_cache = {}


def _split_multi_waits(nc, maxw=1):
    """Walrus in this container accepts only one sync-wait per
    instruction; move extra waits onto preceding same-engine NoOps."""
    from concourse import mybir

    n_split = 0
    for fn in nc.m.functions:
        for bb in fn.blocks:
            out = []
            changed = False
            for inst in bb.instructions:
                si = inst.sync_info
                waits = list(si.on_wait or []) if si is not None else []
                if len(waits) > maxw:
                    changed = True
                    n_split += 1
                    head, tail = waits[:-maxw], waits[-maxw:]
                    for j in range(0, len(head), maxw):
                        nop = mybir.InstNoOp(
                            name=f"{inst.name}-wsplit{j}", ins=[], outs=[]
                        )
                        nop.engine = inst.engine
                        nop.sync_info = mybir.SyncInfo(
                            on_wait=head[j : j + maxw], on_update=[]
                        )
                        out.append(nop)
                    si.on_wait = tail
                out.append(inst)
            if changed:
                bb.instructions = out
    return n_split


def _build_program():
    import concourse.bass as bass
    import concourse.tile as tile
    from concourse import mybir
    from concourse.masks import make_identity

    F32R = mybir.dt.float32r
    F32 = mybir.dt.float32
    BF16 = mybir.dt.bfloat16
    Exp = mybir.ActivationFunctionType.Exp
    Mult = mybir.AluOpType.mult
    Add = mybir.AluOpType.add

    nc = bass.Bass(target_bir_lowering=False)

    xT = nc.dram_tensor("xT", [E, S], F32R, kind="ExternalInput")
    wq = nc.dram_tensor("wq", [E, R * D], F32R, kind="ExternalInput")
    wk = nc.dram_tensor("wk", [E, D], F32R, kind="ExternalInput")
    wv = nc.dram_tensor("wv", [E, D], F32R, kind="ExternalInput")
    wo = nc.dram_tensor("wo", [R * D, E], F32R, kind="ExternalInput")
    bqv = nc.dram_tensor("bqv", [R * D], F32, kind="ExternalInput")
    bkv = nc.dram_tensor("bkv", [D], F32, kind="ExternalInput")
    bvv = nc.dram_tensor("bvv", [D], F32, kind="ExternalInput")
    otd = nc.dram_tensor("ot", [E, S], F32, kind="ExternalOutput")

    xTr = xT.rearrange("(o p) m -> p o m", p=128)
    wqr = wq.rearrange("(o p) m -> p o m", p=128)
    wkr = wk.rearrange("(o p) m -> p o m", p=128)
    wvr = wv.rearrange("(o p) m -> p o m", p=128)
    wor = wo.rearrange("(o p) m -> p o m", p=128)

    with tile.TileContext(nc) as tc:
        import contextlib

        with contextlib.ExitStack() as ctx:
            consts = ctx.enter_context(tc.tile_pool(name="consts", bufs=1))
            qkvt = ctx.enter_context(tc.tile_pool(name="qkvt", bufs=1))

            ident_f = consts.tile([128, 128], F32)
            make_identity(nc, ident_f)
            ident = consts.tile([128, 128], F32R)
            nc.vector.tensor_copy(ident, ident_f)
            ones_bf = consts.tile([128, 128], BF16)
            nc.gpsimd.memset(ones_bf, 1.0)
            bq_sb = consts.tile([128, R], F32)
            bk_sb = consts.tile([128, 1], F32)
            bv_sb = consts.tile([128, 1], F32)

            QT = qkvt.tile([128, R, S], F32R)    # QT[d, h, s]
            KT = qkvt.tile([128, S], F32R)       # KT[d, t]
            V = qkvt.tile([128, ST, D], BF16)    # V[t%128, tt, d]

            # ---- phase 1: QKV^T projections + V transpose ----
            with tc.tile_pool(name="vt", bufs=1) as vtpool, \
                 tc.tile_pool(name="wts", bufs=1) as wpool, \
                 tc.tile_pool(name="xts", bufs=3) as xtpool, \
                 tc.tile_pool(name="ps1", bufs=7, space="PSUM") as ps1, \
                 tc.tile_pool(name="psv", bufs=1, space="PSUM") as psv:
                VT = vtpool.tile([128, S], F32R)
                wq_sb = wpool.tile([128, ET, R * D], F32R)
                wk_sb = wpool.tile([128, ET, D], F32R)
                wv_sb = wpool.tile([128, ET, D], F32R)
                # e-granular DMAs for the first quarter so the first
                # matmuls unblock asap; remaining quarters spread over the
                # SP and ACT DGE queues so neither queue serializes >3MB
                for e in range(4):
                    sl = slice(e, e + 1)
                    nc.sync.dma_start(wq_sb[:, sl], wqr[:, sl])
                    nc.scalar.dma_start(wk_sb[:, sl], wkr[:, sl])
                    nc.scalar.dma_start(wv_sb[:, sl], wvr[:, sl])
                for q in range(1, 4):
                    sl = slice(q * 4, q * 4 + 4)
                    eng = nc.scalar if q == 2 else nc.sync
                    eng.dma_start(wq_sb[:, sl], wqr[:, sl])
                    nc.scalar.dma_start(wk_sb[:, sl], wkr[:, sl])
                    nc.scalar.dma_start(wv_sb[:, sl], wvr[:, sl])
                # biases are tiny and needed late; issue after the weights
                nc.sync.dma_start(bq_sb, bqv.rearrange("(o p) -> p o", p=128))
                nc.sync.dma_start(bk_sb, bkv.rearrange("(o p) -> p o", p=128))
                nc.sync.dma_start(bv_sb, bvv.rearrange("(o p) -> p o", p=128))

                def transposes(sc):
                    tps = psv.tile([128, 512], F32R, tag="pv", name="tps")
                    for i in range(4):
                        tt = sc * 4 + i
                        nc.tensor.transpose(
                            tps[:, i * 128 : (i + 1) * 128],
                            VT[:, tt * 128 : (tt + 1) * 128],
                            ident,
                        )
                    for i in range(4):
                        nc.vector.tensor_copy(
                            V[:, sc * 4 + i], tps[:, i * 128 : (i + 1) * 128]
                        )

                for sc in range(SC):
                    cs = slice(sc * 512, (sc + 1) * 512)
                    po = [ps1.tile([128, 512], F32, tag="p1", name="po")
                          for _ in range(R + 2)]
                    for eq in range(4):
                        xq = xtpool.tile([128, 4, 512], F32R, tag="xt")
                        if sc == 0 and eq == 0:
                            # e-granular so the first matmul starts early
                            for i in range(4):
                                nc.gpsimd.dma_start(
                                    xq[:, i : i + 1], xTr[:, i : i + 1, cs]
                                )
                        else:
                            nc.gpsimd.dma_start(
                                xq, xTr[:, eq * 4 : eq * 4 + 4, cs]
                            )
                        for i in range(4):
                            e = eq * 4 + i
                            for ot in range(R + 2):
                                if ot < R:
                                    lhsT = wq_sb[:, e, ot * 128 : (ot + 1) * 128]
                                elif ot == R:
                                    lhsT = wk_sb[:, e]
                                else:
                                    lhsT = wv_sb[:, e]
                                nc.tensor.matmul(
                                    po[ot], lhsT, xq[:, i],
                                    start=(e == 0), stop=(e == ET - 1),
                                )
                        if eq == 1 and sc > 0:
                            # previous chunk's V rows are long since
                            # drained; transpose them here so PE never
                            # waits on the ACT drain queue
                            transposes(sc - 1)
                    # drains; for the last chunk emit V first so its
                    # transposes (right below) wait minimally
                    drains = [(VT[:, cs], po[R + 1], bv_sb[:, 0:1]),
                              (KT[:, cs], po[R], bk_sb[:, 0:1])]
                    qdr = [(QT[:, ot, cs], po[ot], bq_sb[:, ot : ot + 1])
                           for ot in range(R)]
                    order = drains + qdr if sc == SC - 1 else qdr + drains[::-1]
                    for dst, src, bias in order:
                        nc.scalar.add(dst, src, bias)
                transposes(SC - 1)

            # ---- phase 2: attention, software-pipelined ----
            p23 = ctx.enter_context(tc.tile_pool(name="p23", bufs=1))
            outT = p23.tile([128, R, S], F32R)  # normalized attn outT[d, h, s]
            wo_sb = p23.tile([128, R, E], F32R)
            for q in range(4):
                nc.sync.dma_start(wo_sb[:, q], wor[:, q])

            with tc.tile_pool(name="ps_av", bufs=1, space="PSUM") as ps_av, \
                 tc.tile_pool(name="probs", bufs=8) as probs_pool, \
                 tc.tile_pool(name="accp", bufs=2) as accp, \
                 tc.tile_pool(name="avsb", bufs=2) as avsb, \
                 tc.tile_pool(name="rcp", bufs=2) as rcp:

                pss_t = {}   # j -> scores psum tile
                acc_t = {}   # blk -> (dve_acc, pool_acc) bf16 accumulators
                av_t = {}    # blk -> AV psum tile
                avsb_t = {}  # blk -> AV sbuf drain tile
                pt_tail = {}  # blk -> tail prob tiles summed directly on PE

                def finalize(b, sums_pool, sums_tag):
                    """Deferred per-block tail: reduce the prob accumulators
                    (+ the tail prob tiles, summed directly on PE to offload
                    DVE/Pool), reciprocal, and normalize into outT.  Runs 4
                    tiles into the next block so nothing on PE ever waits
                    for it."""
                    h, pr = b // NPAIR, b % NPAIR
                    q0 = pr * 1024
                    acc_d, acc_p = acc_t.pop(b)
                    srcs = [acc_d, acc_p] + pt_tail.pop(b)
                    sums = sums_pool.tile(
                        [128, 1024], F32, tag=sums_tag, name="sums"
                    )
                    for hf in range(2):
                        hs = slice(hf * 512, (hf + 1) * 512)
                        for si, src in enumerate(srcs):
                            nc.tensor.matmul(
                                sums[:, hs], ones_bf, src[:, hs],
                                start=(si == 0), stop=(si == len(srcs) - 1),
                            )
                    rc = rcp.tile([128, 1024], F32, tag="rc", name="rc")
                    nc.vector.reciprocal(rc, sums)
                    nc.vector.tensor_tensor(
                        outT[:, h, q0 : q0 + 1024], avsb_t.pop(b), rc, Mult
                    )

                with tc.tile_pool(name="ps_mix", bufs=3, space="PSUM") as ps_mix:
                    for j in range(16 * NBLK + LOOK):
                        if j >= 20 and (j - 20) % 16 == 0 and (j - 20) // 16 < NBLK - 1:
                            finalize((j - 20) // 16, ps_mix, "s")
                        if j < 16 * NBLK:
                            b, tt = j // ST, j % ST
                            h, pr = b // NPAIR, b % NPAIR
                            q0 = pr * 1024
                            pss = ps_mix.tile(
                                [128, 1024], F32, tag="s", name="pss"
                            )
                            kslice = KT[:, tt * 128 : (tt + 1) * 128]
                            for hf in range(2):
                                nc.tensor.matmul(
                                    pss[:, hf * 512 : (hf + 1) * 512],
                                    kslice,
                                    QT[:, h, q0 + hf * 512 : q0 + (hf + 1) * 512],
                                    start=True, stop=True,
                                )
                            pss_t[j] = pss
                        jj = j - LOOK
                        if 0 <= jj < 16 * NBLK:
                            b, tt = jj // ST, jj % ST
                            h, pr = b // NPAIR, b % NPAIR
                            q0 = pr * 1024
                            pt = probs_pool.tile(
                                [128, 1024], BF16, tag="pt", name="pt"
                            )
                            nc.scalar.activation(pt, pss_t.pop(jj), Exp)
                            # denominator accumulation split: DVE gets the
                            # init copy + 5 even tiles, Pool 7 tiles, PE the
                            # last 3 via finalize's ones-matmul — sized so
                            # each engine stays under PE's per-block time
                            if tt == 0:
                                acc_d = accp.tile(
                                    [128, 1024], BF16, tag="accd", name="accd"
                                )
                                acc_p = accp.tile(
                                    [128, 1024], BF16, tag="accp", name="accp"
                                )
                                acc_t[b] = (acc_d, acc_p)
                                pt_tail[b] = []
                                nc.vector.tensor_copy(acc_d, pt)
                                nc.gpsimd.memset(acc_p, 0.0)
                                av_t[b] = ps_av.tile(
                                    [128, 1024], F32, tag="av", name="avp"
                                )
                            elif tt >= ST - 3:
                                pt_tail[b].append(pt)
                            elif tt % 2 == 0:
                                acc_d = acc_t[b][0]
                                nc.vector.tensor_tensor(acc_d, acc_d, pt, Add)
                            else:
                                acc_p = acc_t[b][1]
                                nc.gpsimd.tensor_tensor(acc_p, acc_p, pt, Add)
                            av = av_t[b]
                            for hf in range(2):
                                hs = slice(hf * 512, (hf + 1) * 512)
                                nc.tensor.matmul(
                                    av[:, hs], V[:, tt], pt[:, hs],
                                    start=(tt == 0), stop=(tt == ST - 1),
                                )
                            if tt == ST - 1:
                                # drain AV psum immediately so the next
                                # block's AV start never waits on the
                                # normalize chain
                                asb = avsb.tile(
                                    [128, 1024], F32, tag="avsb", name="asb"
                                )
                                avsb_t[b] = asb
                                nc.vector.tensor_copy(asb, av_t.pop(b))

                # last block's reduction goes through the AV pool so closing
                # ps_mix (and opening phase 3's pool) doesn't wait on it
                finalize(NBLK - 1, ps_av, "av")

                # ---- phase 3: output projection (transposed) ----
                with tc.tile_pool(name="ostage", bufs=4) as ostage, \
                     tc.tile_pool(name="ps_o", bufs=6, space="PSUM") as ps_o:
                    # sc outer: the first tiles only need the pr=0 (even)
                    # blocks, so P3 never waits on the last blocks' deferred
                    # normalize
                    for sc in range(SC):
                        for et in range(ET):
                            ps = ps_o.tile([128, 512], F32, tag="po", name="ps")
                            for h in range(R):
                                nc.tensor.matmul(
                                    ps,
                                    wo_sb[:, h, et * 128 : (et + 1) * 128],
                                    outT[:, h, sc * 512 : (sc + 1) * 512],
                                    start=(h == 0), stop=(h == R - 1),
                                )
                            st = ostage.tile([128, 512], F32, tag="ost", name="st")
                            nc.vector.tensor_copy(st, ps)
                            nc.sync.dma_start(
                                otd[et * 128 : (et + 1) * 128,
                                    sc * 512 : (sc + 1) * 512],
                                st,
                            )

    _split_multi_waits(nc)
    return nc


def _prepare(x, Wq, bq, Wk, bk, Wv, bv, Wo, bo):
    """Host-side sharding: build per-core input maps."""
    x = np.asarray(x, dtype=np.float32)
    Wq = np.asarray(Wq, dtype=np.float32)
    bq = np.asarray(bq, dtype=np.float32)
    Wk = np.asarray(Wk, dtype=np.float32)
    bk = np.asarray(bk, dtype=np.float32)
    Wv = np.asarray(Wv, dtype=np.float32)
    bv = np.asarray(bv, dtype=np.float32)
    Wo = np.asarray(Wo, dtype=np.float32)

    isd = np.float32(1.0 / np.sqrt(D))
    xTs = [np.ascontiguousarray(x[b].T) for b in range(B)]
    in_maps = []
    for core in range(8):
        b, g = divmod(core, G)
        in_maps.append({
            "xT": xTs[b],
            "wq": np.ascontiguousarray(Wq[:, g * R * D : (g + 1) * R * D]) * isd,
            "wk": np.ascontiguousarray(Wk[:, g * D : (g + 1) * D]),
            "wv": np.ascontiguousarray(Wv[:, g * D : (g + 1) * D]),
            "wo": np.ascontiguousarray(Wo[g * R * D : (g + 1) * R * D, :]),
            "bqv": bq[g * R * D : (g + 1) * R * D] * isd,
            "bkv": bk[g * D : (g + 1) * D],
            "bvv": bv[g * D : (g + 1) * D],
        })
    return in_maps


def _gather(results, bo):
    bo = np.asarray(bo, dtype=np.float32)
    out = np.empty((B, S, E), dtype=np.float32)
    for b in range(B):
        acc = results[b * G]["ot"].copy()
        for g in range(1, G):
            acc += results[b * G + g]["ot"]
        out[b] = acc.T + bo
    return out


def kernel(x, Wq, bq, Wk, bk, Wv, bv, Wo, bo):
    from concourse.bass_utils import run_bass_kernel_spmd

    if "nc" not in _cache:
        _cache["nc"] = _build_program()
    nc = _cache["nc"]
    in_maps = _prepare(x, Wq, bq, Wk, bk, Wv, bv, Wo, bo)
    res = run_bass_kernel_spmd(nc, in_maps, core_ids=list(range(8)))
    return _gather(res.results, bo)


# revision 22
# speedup vs baseline: 1.3672x; 1.0382x over previous
"""GQA attention kernel for 8 Trainium2 NeuronCores.

Sharding: core = (batch b, kv_group g), b in {0,1}, g in {0..3}.
Each core computes the 4 heads of one KV group for one batch and the
partial output projection for those heads; the host sums the 4 group
partials per batch.  Zero duplicated compute across cores.

v4 design (baseline was 516us):
  - P1 (QKV proj) e-outer with 6 concurrent PSUM accumulation groups;
    e-granular first-quarter DMAs spread across the SP/ACT/Pool DGE
    queues so the first matmul starts ~12us in and PE never waits on
    DMA; V transposed through a side PSUM bank inside the loop.
  - P2 (attention) software-pipelined with lookahead-2 scores in a
    3-deep PSUM rotation so PE never stalls on the ACT exp chain.
  - softmax denominators: probs written bf16; accumulation split
    across DVE (5 tiles + init copy), Pool (7 tiles), and PE
    (3 tail tiles via the final ones-matmul), sized from measured
    per-op costs so every engine stays under PE's per-block time.
  - per-block normalize (reciprocal + multiply) deferred 4 tiles into
    the next block, AV PSUM drained immediately by DVE, so no PE
    instruction ever waits on the normalize chain; the last block's
    reduction uses the AV PSUM pool so phase 3's PSUM pool opens
    without waiting on it.
  - numerics: scores/Q/K/weights stay fp32r; only probs/V/acc are
    bf16 (validated 2.1e-3 max rel err vs 2e-2 budget).
"""

import numpy as np

# problem shape (hardcoded per contract)
B, S, E = 2, 2048, 2048
H, G, D = 16, 4, 128
R = H // G          # heads per kv group = 4
KV = G * D          # 512
ST = S // 128       # 16 t-tiles
ET = E // 128       # 16 e-tiles
SC = S // 512       # 4 s-chunks
NPAIR = S // 1024   # 2 q-chunk pairs
NBLK = R * NPAIR    # 8 attention blocks per core
LOOK = 2            # scores lookahead (PSUM rotation depth - 1)

_cache = {}


def _split_multi_waits(nc, maxw=1):
    """Walrus in this container accepts only one sync-wait per
    instruction; move extra waits onto preceding same-engine NoOps."""
    from concourse import mybir

    n_split = 0
    for fn in nc.m.functions:
        for bb in fn.blocks:
            out = []
            changed = False
            for inst in bb.instructions:
                si = inst.sync_info
                waits = list(si.on_wait or []) if si is not None else []
                if len(waits) > maxw:
                    changed = True
                    n_split += 1
                    head, tail = waits[:-maxw], waits[-maxw:]
                    for j in range(0, len(head), maxw):
                        nop = mybir.InstNoOp(
                            name=f"{inst.name}-wsplit{j}", ins=[], outs=[]
                        )
                        nop.engine = inst.engine
                        nop.sync_info = mybir.SyncInfo(
                            on_wait=head[j : j + maxw], on_update=[]
                        )
                        out.append(nop)
                    si.on_wait = tail
                out.append(inst)
            if changed:
                bb.instructions = out
    return n_split


def _build_program():
    import concourse.bass as bass
    import concourse.tile as tile
    from concourse import mybir
    from concourse.masks import make_identity

    F32R = mybir.dt.float32r
    F32 = mybir.dt.float32
    BF16 = mybir.dt.bfloat16
    Exp = mybir.ActivationFunctionType.Exp
    Mult = mybir.AluOpType.mult
    Add = mybir.AluOpType.add

    nc = bass.Bass(target_bir_lowering=False)

    xT = nc.dram_tensor("xT", [E, S], F32R, kind="ExternalInput")
    wq = nc.dram_tensor("wq", [E, R * D], F32R, kind="ExternalInput")
    wk = nc.dram_tensor("wk", [E, D], F32R, kind="ExternalInput")
    wv = nc.dram_tensor("wv", [E, D], F32R, kind="ExternalInput")
    wo = nc.dram_tensor("wo", [R * D, E], F32R, kind="ExternalInput")
    bqv = nc.dram_tensor("bqv", [R * D], F32, kind="ExternalInput")
    bkv = nc.dram_tensor("bkv", [D], F32, kind="ExternalInput")
    bvv = nc.dram_tensor("bvv", [D], F32, kind="ExternalInput")
    otd = nc.dram_tensor("ot", [E, S], F32, kind="ExternalOutput")

    xTr = xT.rearrange("(o p) m -> p o m", p=128)
    wqr = wq.rearrange("(o p) m -> p o m", p=128)
    wkr = wk.rearrange("(o p) m -> p o m", p=128)
    wvr = wv.rearrange("(o p) m -> p o m", p=128)
    wor = wo.rearrange("(o p) m -> p o m", p=128)

    with tile.TileContext(nc) as tc:
        import contextlib

        with contextlib.ExitStack() as ctx:
            consts = ctx.enter_context(tc.tile_pool(name="consts", bufs=1))
            qkvt = ctx.enter_context(tc.tile_pool(name="qkvt", bufs=1))

            ident_f = consts.tile([128, 128], F32)
            make_identity(nc, ident_f)
            ident = consts.tile([128, 128], F32R)
            nc.vector.tensor_copy(ident, ident_f)
            ones_bf = consts.tile([128, 128], BF16)
            nc.gpsimd.memset(ones_bf, 1.0)
            bq_sb = consts.tile([128, R], F32)
            bk_sb = consts.tile([128, 1], F32)
            bv_sb = consts.tile([128, 1], F32)

            QT = qkvt.tile([128, R, S], F32R)    # QT[d, h, s]
            KT = qkvt.tile([128, S], F32R)       # KT[d, t]
            V = qkvt.tile([128, ST, D], BF16)    # V[t%128, tt, d]

            # ---- phase 1: QKV^T projections + V transpose ----
            with tc.tile_pool(name="vt", bufs=1) as vtpool, \
                 tc.tile_pool(name="wts", bufs=1) as wpool, \
                 tc.tile_pool(name="xts", bufs=3) as xtpool, \
                 tc.tile_pool(name="ps1", bufs=7, space="PSUM") as ps1, \
                 tc.tile_pool(name="psv", bufs=1, space="PSUM") as psv:
                VT = vtpool.tile([128, S], F32R)
                wq_sb = wpool.tile([128, ET, R * D], F32R)
                wk_sb = wpool.tile([128, ET, D], F32R)
                wv_sb = wpool.tile([128, ET, D], F32R)
                # e-granular DMAs for the first quarter so the first
                # matmuls unblock asap; remaining quarters spread over the
                # SP and ACT DGE queues so neither queue serializes >3MB
                for e in range(4):
                    sl = slice(e, e + 1)
                    nc.sync.dma_start(wq_sb[:, sl], wqr[:, sl])
                    nc.scalar.dma_start(wk_sb[:, sl], wkr[:, sl])
                    nc.scalar.dma_start(wv_sb[:, sl], wvr[:, sl])
                for q in range(1, 4):
                    sl = slice(q * 4, q * 4 + 4)
                    nc.sync.dma_start(wq_sb[:, sl], wqr[:, sl])
                bulk = slice(4, ET)
                nc.scalar.dma_start(wk_sb[:, bulk], wkr[:, bulk])
                nc.scalar.dma_start(wv_sb[:, bulk], wvr[:, bulk])
                # biases are tiny and needed late; issue after the weights
                nc.sync.dma_start(bq_sb, bqv.rearrange("(o p) -> p o", p=128))
                nc.sync.dma_start(bk_sb, bkv.rearrange("(o p) -> p o", p=128))
                nc.sync.dma_start(bv_sb, bvv.rearrange("(o p) -> p o", p=128))

                def transposes(sc):
                    tps = psv.tile([128, 512], F32R, tag="pv", name="tps")
                    for i in range(4):
                        tt = sc * 4 + i
                        nc.tensor.transpose(
                            tps[:, i * 128 : (i + 1) * 128],
                            VT[:, tt * 128 : (tt + 1) * 128],
                            ident,
                        )
                    for i in range(4):
                        nc.vector.tensor_copy(
                            V[:, sc * 4 + i], tps[:, i * 128 : (i + 1) * 128]
                        )

                for sc in range(SC):
                    cs = slice(sc * 512, (sc + 1) * 512)
                    po = [ps1.tile([128, 512], F32, tag="p1", name="po")
                          for _ in range(R + 2)]
                    for eq in range(4):
                        xq = xtpool.tile([128, 4, 512], F32R, tag="xt")
                        if sc == 0 and eq == 0:
                            # e-granular so the first matmul starts early
                            for i in range(4):
                                nc.gpsimd.dma_start(
                                    xq[:, i : i + 1], xTr[:, i : i + 1, cs]
                                )
                        else:
                            nc.gpsimd.dma_start(
                                xq, xTr[:, eq * 4 : eq * 4 + 4, cs]
                            )
                        for i in range(4):
                            e = eq * 4 + i
                            for ot in range(R + 2):
                                if ot < R:
                                    lhsT = wq_sb[:, e, ot * 128 : (ot + 1) * 128]
                                elif ot == R:
                                    lhsT = wk_sb[:, e]
                                else:
                                    lhsT = wv_sb[:, e]
                                nc.tensor.matmul(
                                    po[ot], lhsT, xq[:, i],
                                    start=(e == 0), stop=(e == ET - 1),
                                )
                        if eq == 1 and sc > 0:
                            # previous chunk's V rows are long since
                            # drained; transpose them here so PE never
                            # waits on the ACT drain queue
                            transposes(sc - 1)
                    # drains; for the last chunk emit V first so its
                    # transposes (right below) wait minimally
                    drains = [(VT[:, cs], po[R + 1], bv_sb[:, 0:1]),
                              (KT[:, cs], po[R], bk_sb[:, 0:1])]
                    qdr = [(QT[:, ot, cs], po[ot], bq_sb[:, ot : ot + 1])
                           for ot in range(R)]
                    order = drains + qdr if sc == SC - 1 else qdr + drains[::-1]
                    for dst, src, bias in order:
                        nc.scalar.add(dst, src, bias)
                transposes(SC - 1)

            # ---- phase 2: attention, software-pipelined ----
            p23 = ctx.enter_context(tc.tile_pool(name="p23", bufs=1))
            outT = p23.tile([128, R, S], F32R)  # normalized attn outT[d, h, s]
            wo_sb = p23.tile([128, R, E], F32R)
            for q in range(4):
                nc.sync.dma_start(wo_sb[:, q], wor[:, q])

            with tc.tile_pool(name="ps_av", bufs=1, space="PSUM") as ps_av, \
                 tc.tile_pool(name="probs", bufs=10) as probs_pool, \
                 tc.tile_pool(name="accp", bufs=2) as accp, \
                 tc.tile_pool(name="avsb", bufs=2) as avsb, \
                 tc.tile_pool(name="smsb", bufs=2) as smsb, \
                 tc.tile_pool(name="rcp", bufs=2) as rcp:

                pss_t = {}   # j -> scores psum tile
                acc_t = {}   # blk -> (dve_acc, pool_acc) bf16 accumulators
                av_t = {}    # blk -> AV psum tile
                avsb_t = {}  # blk -> AV sbuf drain tile
                pt_tail = {}  # blk -> tail prob tiles summed directly on PE

                def finalize(b, sums_pool, sums_tag):
                    """Deferred per-block tail: reduce the prob accumulators
                    (+ the tail prob tiles, summed directly on PE to offload
                    DVE/Pool), reciprocal, and normalize into outT.  Runs 5
                    tiles into the next block so nothing on PE ever waits
                    for it.  The sums PSUM tile is drained to SBUF by a fast
                    DVE copy before the slow reciprocal reads it, so the
                    PSUM slot recycles in ~0.7us instead of ~6.5us (the
                    reciprocal-blocks-scores WAR stall)."""
                    h, pr = b // NPAIR, b % NPAIR
                    q0 = pr * 1024
                    acc_d, acc_p = acc_t.pop(b)
                    srcs = [acc_d, acc_p] + pt_tail.pop(b)
                    sums = sums_pool.tile(
                        [128, 1024], F32, tag=sums_tag, name="sums"
                    )
                    for hf in range(2):
                        hs = slice(hf * 512, (hf + 1) * 512)
                        for si, src in enumerate(srcs):
                            nc.tensor.matmul(
                                sums[:, hs], ones_bf, src[:, hs],
                                start=(si == 0), stop=(si == len(srcs) - 1),
                            )
                    ssb = smsb.tile([128, 1024], F32, tag="ssb", name="ssb")
                    nc.vector.tensor_copy(ssb, sums)
                    rc = rcp.tile([128, 1024], F32, tag="rc", name="rc")
                    nc.vector.reciprocal(rc, ssb)
                    # normalize on Pool (all-SBUF operands) to keep DVE
                    # under PE's per-block time
                    nc.gpsimd.tensor_tensor(
                        outT[:, h, q0 : q0 + 1024], avsb_t.pop(b), rc, Mult
                    )

                with tc.tile_pool(name="ps_mix", bufs=3, space="PSUM") as ps_mix:
                    for j in range(16 * NBLK + LOOK):
                        if j < 16 * NBLK:
                            b, tt = j // ST, j % ST
                            h, pr = b // NPAIR, b % NPAIR
                            q0 = pr * 1024
                            pss = ps_mix.tile(
                                [128, 1024], F32, tag="s", name="pss"
                            )
                            kslice = KT[:, tt * 128 : (tt + 1) * 128]
                            for hf in range(2):
                                nc.tensor.matmul(
                                    pss[:, hf * 512 : (hf + 1) * 512],
                                    kslice,
                                    QT[:, h, q0 + hf * 512 : q0 + (hf + 1) * 512],
                                    start=True, stop=True,
                                )
                            pss_t[j] = pss
                        jj = j - LOOK
                        if 0 <= jj < 16 * NBLK:
                            b, tt = jj // ST, jj % ST
                            h, pr = b // NPAIR, b % NPAIR
                            q0 = pr * 1024
                            pt = probs_pool.tile(
                                [128, 1024], BF16, tag="pt", name="pt"
                            )
                            nc.scalar.activation(pt, pss_t.pop(jj), Exp)
                            # denominator accumulation split, sized from
                            # measured per-op costs so each engine stays
                            # under PE's per-block time: DVE gets the init
                            # copy + 4 even tiles, Pool 6 tiles, PE the last
                            # 5 via finalize's ones-matmul
                            if tt == 0:
                                acc_d = accp.tile(
                                    [128, 1024], BF16, tag="accd", name="accd"
                                )
                                acc_p = accp.tile(
                                    [128, 1024], BF16, tag="accp", name="accp"
                                )
                                acc_t[b] = (acc_d, acc_p)
                                pt_tail[b] = []
                                nc.vector.tensor_copy(acc_d, pt)
                                nc.gpsimd.memset(acc_p, 0.0)
                                av_t[b] = ps_av.tile(
                                    [128, 1024], F32, tag="av", name="avp"
                                )
                            elif tt >= ST - 5:
                                pt_tail[b].append(pt)
                            elif tt % 2 == 0 and tt <= 8:
                                acc_d = acc_t[b][0]
                                nc.vector.tensor_tensor(acc_d, acc_d, pt, Add)
                            else:
                                acc_p = acc_t[b][1]
                                nc.gpsimd.tensor_tensor(acc_p, acc_p, pt, Add)
                            av = av_t[b]
                            for hf in range(2):
                                hs = slice(hf * 512, (hf + 1) * 512)
                                nc.tensor.matmul(
                                    av[:, hs], V[:, tt], pt[:, hs],
                                    start=(tt == 0), stop=(tt == ST - 1),
                                )
                            if tt == ST - 1:
                                # drain AV psum immediately so the next
                                # block's AV start never waits on the
                                # normalize chain
                                asb = avsb.tile(
                                    [128, 1024], F32, tag="avsb", name="asb"
                                )
                                avsb_t[b] = asb
                                nc.vector.tensor_copy(asb, av_t.pop(b))
                        # per-block tail emitted after this iteration's
                        # scores so the exp pipeline is never starved by
                        # the 14-matmul sums burst
                        if j >= 21 and (j - 21) % 16 == 0 and (j - 21) // 16 < NBLK - 1:
                            finalize((j - 21) // 16, ps_mix, "s")

                # last block's reduction goes through the AV pool so closing
                # ps_mix (and opening phase 3's pool) doesn't wait on it
                finalize(NBLK - 1, ps_av, "av")

                # ---- phase 3: output projection (transposed) ----
                with tc.tile_pool(name="ostage", bufs=4) as ostage, \
                     tc.tile_pool(name="ps_o", bufs=6, space="PSUM") as ps_o:
                    # sc outer: the first tiles only need the pr=0 (even)
                    # blocks, so P3 never waits on the last blocks' deferred
                    # normalize
                    for sc in range(SC):
                        for et in range(ET):
                            ps = ps_o.tile([128, 512], F32, tag="po", name="ps")
                            for h in range(R):
                                nc.tensor.matmul(
                                    ps,
                                    wo_sb[:, h, et * 128 : (et + 1) * 128],
                                    outT[:, h, sc * 512 : (sc + 1) * 512],
                                    start=(h == 0), stop=(h == R - 1),
                                )
                            st = ostage.tile([128, 512], F32, tag="ost", name="st")
                            nc.vector.tensor_copy(st, ps)
                            nc.sync.dma_start(
                                otd[et * 128 : (et + 1) * 128,
                                    sc * 512 : (sc + 1) * 512],
                                st,
                            )

    _split_multi_waits(nc)
    return nc


def _prepare(x, Wq, bq, Wk, bk, Wv, bv, Wo, bo):
    """Host-side sharding: build per-core input maps."""
    x = np.asarray(x, dtype=np.float32)
    Wq = np.asarray(Wq, dtype=np.float32)
    bq = np.asarray(bq, dtype=np.float32)
    Wk = np.asarray(Wk, dtype=np.float32)
    bk = np.asarray(bk, dtype=np.float32)
    Wv = np.asarray(Wv, dtype=np.float32)
    bv = np.asarray(bv, dtype=np.float32)
    Wo = np.asarray(Wo, dtype=np.float32)

    isd = np.float32(1.0 / np.sqrt(D))
    xTs = [np.ascontiguousarray(x[b].T) for b in range(B)]
    in_maps = []
    for core in range(8):
        b, g = divmod(core, G)
        in_maps.append({
            "xT": xTs[b],
            "wq": np.ascontiguousarray(Wq[:, g * R * D : (g + 1) * R * D]) * isd,
            "wk": np.ascontiguousarray(Wk[:, g * D : (g + 1) * D]),
            "wv": np.ascontiguousarray(Wv[:, g * D : (g + 1) * D]),
            "wo": np.ascontiguousarray(Wo[g * R * D : (g + 1) * R * D, :]),
            "bqv": bq[g * R * D : (g + 1) * R * D] * isd,
            "bkv": bk[g * D : (g + 1) * D],
            "bvv": bv[g * D : (g + 1) * D],
        })
    return in_maps


def _gather(results, bo):
    bo = np.asarray(bo, dtype=np.float32)
    out = np.empty((B, S, E), dtype=np.float32)
    for b in range(B):
        acc = results[b * G]["ot"].copy()
        for g in range(1, G):
            acc += results[b * G + g]["ot"]
        out[b] = acc.T + bo
    return out


def kernel(x, Wq, bq, Wk, bk, Wv, bv, Wo, bo):
    from concourse.bass_utils import run_bass_kernel_spmd

    if "nc" not in _cache:
        _cache["nc"] = _build_program()
    nc = _cache["nc"]
    in_maps = _prepare(x, Wq, bq, Wk, bk, Wv, bv, Wo, bo)
    res = run_bass_kernel_spmd(nc, in_maps, core_ids=list(range(8)))
    return _gather(res.results, bo)


# revision 26
# speedup vs baseline: 1.4570x; 1.0657x over previous
"""GQA attention kernel for 8 Trainium2 NeuronCores.

Sharding: core = (batch b, kv_group g), b in {0,1}, g in {0..3}.
Each core computes the 4 heads of one KV group for one batch and the
partial output projection for those heads; the host sums the 4 group
partials per batch.  Zero duplicated compute across cores.

v4 design (baseline was 516us):
  - P1 (QKV proj) e-outer with 6 concurrent PSUM accumulation groups;
    e-granular first-quarter DMAs spread across the SP/ACT/Pool DGE
    queues so the first matmul starts ~12us in and PE never waits on
    DMA; V transposed through a side PSUM bank inside the loop.
  - P2 (attention) software-pipelined with lookahead-2 scores in a
    3-deep PSUM rotation so PE never stalls on the ACT exp chain.
  - softmax denominators: probs written bf16; accumulation split
    across DVE (5 tiles + init copy), Pool (7 tiles), and PE
    (3 tail tiles via the final ones-matmul), sized from measured
    per-op costs so every engine stays under PE's per-block time.
  - per-block normalize (reciprocal + multiply) deferred 4 tiles into
    the next block, AV PSUM drained immediately by DVE, so no PE
    instruction ever waits on the normalize chain; the last block's
    reduction uses the AV PSUM pool so phase 3's PSUM pool opens
    without waiting on it.
  - numerics: scores/Q/K/weights stay fp32r; only probs/V/acc are
    bf16 (validated 2.1e-3 max rel err vs 2e-2 budget).
"""

import numpy as np

# problem shape (hardcoded per contract)
B, S, E = 2, 2048, 2048
H, G, D = 16, 4, 128
R = H // G          # heads per kv group = 4
KV = G * D          # 512
ST = S // 128       # 16 t-tiles
ET = E // 128       # 16 e-tiles
SC = S // 512       # 4 s-chunks
NPAIR = S // 1024   # 2 q-chunk pairs
NBLK = R * NPAIR    # 8 attention blocks per core
LOOK = 2            # scores lookahead (PSUM rotation depth - 1)

_cache = {}


def _split_multi_waits(nc, maxw=1):
    """Walrus in this container accepts only one sync-wait per
    instruction; move extra waits onto preceding same-engine NoOps."""
    from concourse import mybir

    n_split = 0
    for fn in nc.m.functions:
        for bb in fn.blocks:
            out = []
            changed = False
            for inst in bb.instructions:
                si = inst.sync_info
                waits = list(si.on_wait or []) if si is not None else []
                if len(waits) > maxw:
                    changed = True
                    n_split += 1
                    head, tail = waits[:-maxw], waits[-maxw:]
                    for j in range(0, len(head), maxw):
                        nop = mybir.InstNoOp(
                            name=f"{inst.name}-wsplit{j}", ins=[], outs=[]
                        )
                        nop.engine = inst.engine
                        nop.sync_info = mybir.SyncInfo(
                            on_wait=head[j : j + maxw], on_update=[]
                        )
                        out.append(nop)
                    si.on_wait = tail
                out.append(inst)
            if changed:
                bb.instructions = out
    return n_split


def _build_program():
    import concourse.bass as bass
    import concourse.tile as tile
    from concourse import mybir
    from concourse.masks import make_identity

    F32R = mybir.dt.float32r
    F32 = mybir.dt.float32
    BF16 = mybir.dt.bfloat16
    Exp = mybir.ActivationFunctionType.Exp
    Mult = mybir.AluOpType.mult
    Add = mybir.AluOpType.add

    nc = bass.Bass(target_bir_lowering=False)

    xT = nc.dram_tensor("xT", [E, S], F32R, kind="ExternalInput")
    wq = nc.dram_tensor("wq", [E, R * D], F32R, kind="ExternalInput")
    wk = nc.dram_tensor("wk", [E, D], F32R, kind="ExternalInput")
    wv = nc.dram_tensor("wv", [E, D], F32R, kind="ExternalInput")
    wo = nc.dram_tensor("wo", [R * D, E], F32R, kind="ExternalInput")
    bqv = nc.dram_tensor("bqv", [R * D], F32, kind="ExternalInput")
    bkv = nc.dram_tensor("bkv", [D], F32, kind="ExternalInput")
    bvv = nc.dram_tensor("bvv", [D], F32, kind="ExternalInput")
    otd = nc.dram_tensor("ot", [E, S], F32, kind="ExternalOutput")

    xTr = xT.rearrange("(o p) m -> p o m", p=128)
    wqr = wq.rearrange("(o p) m -> p o m", p=128)
    wkr = wk.rearrange("(o p) m -> p o m", p=128)
    wvr = wv.rearrange("(o p) m -> p o m", p=128)
    wor = wo.rearrange("(o p) m -> p o m", p=128)

    with tile.TileContext(nc) as tc:
        import contextlib

        with contextlib.ExitStack() as ctx:
            consts = ctx.enter_context(tc.tile_pool(name="consts", bufs=1))
            qkvt = ctx.enter_context(tc.tile_pool(name="qkvt", bufs=1))

            ident_f = consts.tile([128, 128], F32)
            make_identity(nc, ident_f)
            ident = consts.tile([128, 128], F32R)
            nc.vector.tensor_copy(ident, ident_f)
            ones_bf = consts.tile([128, 128], BF16)
            nc.gpsimd.memset(ones_bf, 1.0)
            bq_sb = consts.tile([128, R], F32)
            bk_sb = consts.tile([128, 1], F32)
            bv_sb = consts.tile([128, 1], F32)

            QT = qkvt.tile([128, R, S], F32R)    # QT[d, h, s]
            KT = qkvt.tile([128, S], F32R)       # KT[d, t]
            V = qkvt.tile([128, ST, D], BF16)    # V[t%128, tt, d]

            # ---- phase 1: QKV^T projections + V transpose ----
            with tc.tile_pool(name="vt", bufs=1) as vtpool, \
                 tc.tile_pool(name="wts", bufs=1) as wpool, \
                 tc.tile_pool(name="xts", bufs=3) as xtpool, \
                 tc.tile_pool(name="ps1", bufs=7, space="PSUM") as ps1, \
                 tc.tile_pool(name="psv", bufs=1, space="PSUM") as psv:
                VT = vtpool.tile([128, S], F32R)
                wq_sb = wpool.tile([128, ET, R * D], F32R)
                wk_sb = wpool.tile([128, ET, D], F32R)
                wv_sb = wpool.tile([128, ET, D], F32R)
                # e-granular DMAs for the first quarter so the first
                # matmuls unblock asap; remaining quarters spread over the
                # SP and ACT DGE queues so neither queue serializes >3MB
                nc.sync.dma_start(wq_sb[:, 0:1], wqr[:, 0:1])
                nc.scalar.dma_start(wk_sb[:, 0:4], wkr[:, 0:4])
                nc.scalar.dma_start(wv_sb[:, 0:4], wvr[:, 0:4])
                nc.sync.dma_start(wq_sb[:, 1:4], wqr[:, 1:4])
                for q in range(1, 4):
                    sl = slice(q * 4, q * 4 + 4)
                    nc.sync.dma_start(wq_sb[:, sl], wqr[:, sl])
                for half in (slice(4, 10), slice(10, ET)):
                    nc.scalar.dma_start(wk_sb[:, half], wkr[:, half])
                    nc.scalar.dma_start(wv_sb[:, half], wvr[:, half])
                # biases are tiny and needed late; issue after the weights
                nc.sync.dma_start(bq_sb, bqv.rearrange("(o p) -> p o", p=128))
                nc.sync.dma_start(bk_sb, bkv.rearrange("(o p) -> p o", p=128))
                nc.sync.dma_start(bv_sb, bvv.rearrange("(o p) -> p o", p=128))

                def transposes(sc):
                    tps = psv.tile([128, 512], F32R, tag="pv", name="tps")
                    for i in range(4):
                        tt = sc * 4 + i
                        nc.tensor.transpose(
                            tps[:, i * 128 : (i + 1) * 128],
                            VT[:, tt * 128 : (tt + 1) * 128],
                            ident,
                        )
                    for i in range(4):
                        nc.vector.tensor_copy(
                            V[:, sc * 4 + i], tps[:, i * 128 : (i + 1) * 128]
                        )

                for sc in range(SC):
                    cs = slice(sc * 512, (sc + 1) * 512)
                    po = [ps1.tile([128, 512], F32, tag="p1", name="po")
                          for _ in range(R + 2)]
                    for eq in range(4):
                        xq = xtpool.tile([128, 4, 512], F32R, tag="xt")
                        if sc == 0 and eq == 0:
                            # e-granular so the first matmul starts early
                            for i in range(4):
                                nc.gpsimd.dma_start(
                                    xq[:, i : i + 1], xTr[:, i : i + 1, cs]
                                )
                        else:
                            nc.gpsimd.dma_start(
                                xq, xTr[:, eq * 4 : eq * 4 + 4, cs]
                            )
                        for i in range(4):
                            e = eq * 4 + i
                            for ot in range(R + 2):
                                if ot < R:
                                    lhsT = wq_sb[:, e, ot * 128 : (ot + 1) * 128]
                                elif ot == R:
                                    lhsT = wk_sb[:, e]
                                else:
                                    lhsT = wv_sb[:, e]
                                nc.tensor.matmul(
                                    po[ot], lhsT, xq[:, i],
                                    start=(e == 0), stop=(e == ET - 1),
                                )
                        if eq == 1 and sc > 0:
                            # previous chunk's V rows are long since
                            # drained; transpose them here so PE never
                            # waits on the ACT drain queue
                            transposes(sc - 1)
                    # drains; for the last chunk emit V first so its
                    # transposes (right below) wait minimally
                    drains = [(VT[:, cs], po[R + 1], bv_sb[:, 0:1]),
                              (KT[:, cs], po[R], bk_sb[:, 0:1])]
                    qdr = [(QT[:, ot, cs], po[ot], bq_sb[:, ot : ot + 1])
                           for ot in range(R)]
                    order = drains + qdr if sc == SC - 1 else qdr + drains[::-1]
                    for dst, src, bias in order:
                        nc.scalar.add(dst, src, bias)
                transposes(SC - 1)

            # ---- phase 2: attention, software-pipelined ----
            p23 = ctx.enter_context(tc.tile_pool(name="p23", bufs=1))
            outT = p23.tile([128, R, S], F32R)  # normalized attn outT[d, h, s]
            wo_sb = p23.tile([128, R, E], F32R)
            for q in range(4):
                nc.sync.dma_start(wo_sb[:, q], wor[:, q])

            with tc.tile_pool(name="ps_av", bufs=1, space="PSUM") as ps_av, \
                 tc.tile_pool(name="probs", bufs=10) as probs_pool, \
                 tc.tile_pool(name="accp", bufs=2) as accp, \
                 tc.tile_pool(name="avsb", bufs=2) as avsb, \
                 tc.tile_pool(name="smsb", bufs=2) as smsb, \
                 tc.tile_pool(name="rcp", bufs=2) as rcp:

                pss_t = {}   # j -> scores psum tile
                acc_t = {}   # blk -> (dve_acc, pool_acc) bf16 accumulators
                av_t = {}    # blk -> AV psum tile
                avsb_t = {}  # blk -> AV sbuf drain tile
                pt_tail = {}  # blk -> tail prob tiles summed directly on PE

                def finalize(b, sums_pool, sums_tag):
                    """Deferred per-block tail: reduce the prob accumulators
                    (+ the tail prob tiles, summed directly on PE to offload
                    DVE/Pool), reciprocal, and normalize into outT.  Runs 5
                    tiles into the next block so nothing on PE ever waits
                    for it.  The sums PSUM tile is drained to SBUF by a fast
                    DVE copy before the slow reciprocal reads it, so the
                    PSUM slot recycles in ~0.7us instead of ~6.5us (the
                    reciprocal-blocks-scores WAR stall)."""
                    h, pr = b // NPAIR, b % NPAIR
                    q0 = pr * 1024
                    acc_d, acc_p = acc_t.pop(b)
                    srcs = [acc_d, acc_p] + pt_tail.pop(b)
                    sums = sums_pool.tile(
                        [128, 1024], F32, tag=sums_tag, name="sums"
                    )
                    for hf in range(2):
                        hs = slice(hf * 512, (hf + 1) * 512)
                        for si, src in enumerate(srcs):
                            nc.tensor.matmul(
                                sums[:, hs], ones_bf, src[:, hs],
                                start=(si == 0), stop=(si == len(srcs) - 1),
                            )
                    ssb = smsb.tile([128, 1024], F32, tag="ssb", name="ssb")
                    nc.vector.tensor_copy(ssb, sums)
                    rc = rcp.tile([128, 1024], F32, tag="rc", name="rc")
                    nc.vector.reciprocal(rc, ssb)
                    # normalize on DVE right after the reciprocal so phase 3
                    # never waits behind the next block's Pool adds
                    nc.vector.tensor_tensor(
                        outT[:, h, q0 : q0 + 1024], avsb_t.pop(b), rc, Mult
                    )

                with tc.tile_pool(name="ps_mix", bufs=3, space="PSUM") as ps_mix:
                    for j in range(16 * NBLK + LOOK):
                        if j < 16 * NBLK:
                            b, tt = j // ST, j % ST
                            h, pr = b // NPAIR, b % NPAIR
                            q0 = pr * 1024
                            pss = ps_mix.tile(
                                [128, 1024], F32, tag="s", name="pss"
                            )
                            kslice = KT[:, tt * 128 : (tt + 1) * 128]
                            for hf in range(2):
                                nc.tensor.matmul(
                                    pss[:, hf * 512 : (hf + 1) * 512],
                                    kslice,
                                    QT[:, h, q0 + hf * 512 : q0 + (hf + 1) * 512],
                                    start=True, stop=True,
                                )
                            pss_t[j] = pss
                        jj = j - LOOK
                        if 0 <= jj < 16 * NBLK:
                            b, tt = jj // ST, jj % ST
                            h, pr = b // NPAIR, b % NPAIR
                            q0 = pr * 1024
                            pt = probs_pool.tile(
                                [128, 1024], BF16, tag="pt", name="pt"
                            )
                            nc.scalar.activation(pt, pss_t.pop(jj), Exp)
                            # denominator accumulation split, sized from
                            # measured per-op costs so each engine stays
                            # under PE's per-block time: DVE gets the init
                            # copy + 4 even tiles, Pool 6 tiles, PE the last
                            # 5 via finalize's ones-matmul
                            if tt == 0:
                                acc_d = accp.tile(
                                    [128, 1024], BF16, tag="accd", name="accd"
                                )
                                acc_p = accp.tile(
                                    [128, 1024], BF16, tag="accp", name="accp"
                                )
                                acc_t[b] = (acc_d, acc_p)
                                pt_tail[b] = []
                                nc.vector.tensor_copy(acc_d, pt)
                                nc.gpsimd.memset(acc_p, 0.0)
                                av_t[b] = ps_av.tile(
                                    [128, 1024], F32, tag="av", name="avp"
                                )
                            elif tt >= (ST - 7 if b == NBLK - 1 else ST - 5):
                                # last block hands two extra tiles to PE:
                                # there is no following block to hide the
                                # Pool adds' latency behind
                                pt_tail[b].append(pt)
                            elif tt % 2 == 0 and tt <= 8:
                                acc_d = acc_t[b][0]
                                nc.vector.tensor_tensor(acc_d, acc_d, pt, Add)
                            else:
                                acc_p = acc_t[b][1]
                                nc.gpsimd.tensor_tensor(acc_p, acc_p, pt, Add)
                            av = av_t[b]
                            for hf in range(2):
                                hs = slice(hf * 512, (hf + 1) * 512)
                                nc.tensor.matmul(
                                    av[:, hs], V[:, tt], pt[:, hs],
                                    start=(tt == 0), stop=(tt == ST - 1),
                                )
                            if tt == ST - 1:
                                # drain AV psum immediately so the next
                                # block's AV start never waits on the
                                # normalize chain
                                asb = avsb.tile(
                                    [128, 1024], F32, tag="avsb", name="asb"
                                )
                                avsb_t[b] = asb
                                nc.vector.tensor_copy(asb, av_t.pop(b))
                        # per-block tail emitted after this iteration's
                        # scores so the exp pipeline is never starved by
                        # the 14-matmul sums burst
                        if j >= 21 and (j - 21) % 16 == 0 and (j - 21) // 16 < NBLK - 1:
                            finalize((j - 21) // 16, ps_mix, "s")

                # last block's reduction goes through the AV pool so closing
                # ps_mix (and opening phase 3's pool) doesn't wait on it
                finalize(NBLK - 1, ps_av, "av")

                # ---- phase 3: output projection (transposed) ----
                otr = otd.rearrange("(o p) m -> p o m", p=128)
                with tc.tile_pool(name="ostage", bufs=4) as ostage, \
                     tc.tile_pool(name="ps_o", bufs=6, space="PSUM") as ps_o:
                    # sc outer: the first tiles only need the pr=0 (even)
                    # blocks, so P3 never waits on the last blocks' deferred
                    # normalize; output DMAs go out in et-pairs alternating
                    # between two DGE rings so the ring never backs up
                    for sc in range(SC):
                        cs = slice(sc * 512, (sc + 1) * 512)
                        for etp in range(ET // 2):
                            st = ostage.tile(
                                [128, 2, 512], F32, tag="ost", name="st"
                            )
                            for k in range(2):
                                et = etp * 2 + k
                                ps = ps_o.tile(
                                    [128, 512], F32, tag="po", name="ps"
                                )
                                for h in range(R):
                                    nc.tensor.matmul(
                                        ps,
                                        wo_sb[:, h, et * 128 : (et + 1) * 128],
                                        outT[:, h, cs],
                                        start=(h == 0), stop=(h == R - 1),
                                    )
                                nc.vector.tensor_copy(st[:, k], ps)
                            eng = nc.sync if etp % 2 == 0 else nc.scalar
                            eng.dma_start(
                                otr[:, etp * 2 : etp * 2 + 2, cs], st
                            )

    _split_multi_waits(nc)
    return nc


def _prepare(x, Wq, bq, Wk, bk, Wv, bv, Wo, bo):
    """Host-side sharding: build per-core input maps."""
    x = np.asarray(x, dtype=np.float32)
    Wq = np.asarray(Wq, dtype=np.float32)
    bq = np.asarray(bq, dtype=np.float32)
    Wk = np.asarray(Wk, dtype=np.float32)
    bk = np.asarray(bk, dtype=np.float32)
    Wv = np.asarray(Wv, dtype=np.float32)
    bv = np.asarray(bv, dtype=np.float32)
    Wo = np.asarray(Wo, dtype=np.float32)

    isd = np.float32(1.0 / np.sqrt(D))
    xTs = [np.ascontiguousarray(x[b].T) for b in range(B)]
    in_maps = []
    for core in range(8):
        b, g = divmod(core, G)
        in_maps.append({
            "xT": xTs[b],
            "wq": np.ascontiguousarray(Wq[:, g * R * D : (g + 1) * R * D]) * isd,
            "wk": np.ascontiguousarray(Wk[:, g * D : (g + 1) * D]),
            "wv": np.ascontiguousarray(Wv[:, g * D : (g + 1) * D]),
            "wo": np.ascontiguousarray(Wo[g * R * D : (g + 1) * R * D, :]),
            "bqv": bq[g * R * D : (g + 1) * R * D] * isd,
            "bkv": bk[g * D : (g + 1) * D],
            "bvv": bv[g * D : (g + 1) * D],
        })
    return in_maps


def _gather(results, bo):
    bo = np.asarray(bo, dtype=np.float32)
    out = np.empty((B, S, E), dtype=np.float32)
    for b in range(B):
        acc = results[b * G]["ot"].copy()
        for g in range(1, G):
            acc += results[b * G + g]["ot"]
        out[b] = acc.T + bo
    return out


def kernel(x, Wq, bq, Wk, bk, Wv, bv, Wo, bo):
    from concourse.bass_utils import run_bass_kernel_spmd

    if "nc" not in _cache:
        _cache["nc"] = _build_program()
    nc = _cache["nc"]
    in_maps = _prepare(x, Wq, bq, Wk, bk, Wv, bv, Wo, bo)
    res = run_bass_kernel_spmd(nc, in_maps, core_ids=list(range(8)))
    return _gather(res.results, bo)


# revision 30
# speedup vs baseline: 1.4590x; 1.0014x over previous
"""GQA attention kernel for 8 Trainium2 NeuronCores.

Sharding: core = (batch b, kv_group g), b in {0,1}, g in {0..3}.
Each core computes the 4 heads of one KV group for one batch and the
partial output projection for those heads; the host sums the 4 group
partials per batch.  Zero duplicated compute across cores.

v4 design (baseline was 516us):
  - P1 (QKV proj) e-outer with 6 concurrent PSUM accumulation groups;
    e-granular first-quarter DMAs spread across the SP/ACT/Pool DGE
    queues so the first matmul starts ~12us in and PE never waits on
    DMA; V transposed through a side PSUM bank inside the loop.
  - P2 (attention) software-pipelined with lookahead-2 scores in a
    3-deep PSUM rotation so PE never stalls on the ACT exp chain.
  - softmax denominators: probs written bf16; accumulation split
    across DVE (5 tiles + init copy), Pool (7 tiles), and PE
    (3 tail tiles via the final ones-matmul), sized from measured
    per-op costs so every engine stays under PE's per-block time.
  - per-block normalize (reciprocal + multiply) deferred 4 tiles into
    the next block, AV PSUM drained immediately by DVE, so no PE
    instruction ever waits on the normalize chain; the last block's
    reduction uses the AV PSUM pool so phase 3's PSUM pool opens
    without waiting on it.
  - numerics: scores/Q/K/weights stay fp32r; only probs/V/acc are
    bf16 (validated 2.1e-3 max rel err vs 2e-2 budget).
"""

import numpy as np

# problem shape (hardcoded per contract)
B, S, E = 2, 2048, 2048
H, G, D = 16, 4, 128
R = H // G          # heads per kv group = 4
KV = G * D          # 512
ST = S // 128       # 16 t-tiles
ET = E // 128       # 16 e-tiles
SC = S // 512       # 4 s-chunks
NPAIR = S // 1024   # 2 q-chunk pairs
NBLK = R * NPAIR    # 8 attention blocks per core
LOOK = 2            # scores lookahead (PSUM rotation depth - 1)

_cache = {}


def _split_multi_waits(nc, maxw=1):
    """Walrus in this container accepts only one sync-wait per
    instruction; move extra waits onto preceding same-engine NoOps."""
    from concourse import mybir

    n_split = 0
    for fn in nc.m.functions:
        for bb in fn.blocks:
            out = []
            changed = False
            for inst in bb.instructions:
                si = inst.sync_info
                waits = list(si.on_wait or []) if si is not None else []
                if len(waits) > maxw:
                    changed = True
                    n_split += 1
                    head, tail = waits[:-maxw], waits[-maxw:]
                    for j in range(0, len(head), maxw):
                        nop = mybir.InstNoOp(
                            name=f"{inst.name}-wsplit{j}", ins=[], outs=[]
                        )
                        nop.engine = inst.engine
                        nop.sync_info = mybir.SyncInfo(
                            on_wait=head[j : j + maxw], on_update=[]
                        )
                        out.append(nop)
                    si.on_wait = tail
                out.append(inst)
            if changed:
                bb.instructions = out
    return n_split


def _build_program():
    import concourse.bass as bass
    import concourse.tile as tile
    from concourse import mybir
    from concourse.masks import make_identity

    F32R = mybir.dt.float32r
    F32 = mybir.dt.float32
    BF16 = mybir.dt.bfloat16
    Exp = mybir.ActivationFunctionType.Exp
    Mult = mybir.AluOpType.mult
    Add = mybir.AluOpType.add

    nc = bass.Bass(target_bir_lowering=False)

    xT = nc.dram_tensor("xT", [E, S], F32R, kind="ExternalInput")
    wq = nc.dram_tensor("wq", [E, R * D], F32R, kind="ExternalInput")
    wk = nc.dram_tensor("wk", [E, D], F32R, kind="ExternalInput")
    wv = nc.dram_tensor("wv", [E, D], F32R, kind="ExternalInput")
    wo = nc.dram_tensor("wo", [R * D, E], F32R, kind="ExternalInput")
    bqv = nc.dram_tensor("bqv", [R * D], F32, kind="ExternalInput")
    bkv = nc.dram_tensor("bkv", [D], F32, kind="ExternalInput")
    bvv = nc.dram_tensor("bvv", [D], F32, kind="ExternalInput")
    otd = nc.dram_tensor("ot", [E, S], F32, kind="ExternalOutput")

    xTr = xT.rearrange("(o p) m -> p o m", p=128)
    wqr = wq.rearrange("(o p) m -> p o m", p=128)
    wkr = wk.rearrange("(o p) m -> p o m", p=128)
    wvr = wv.rearrange("(o p) m -> p o m", p=128)
    wor = wo.rearrange("(o p) m -> p o m", p=128)

    with tile.TileContext(nc) as tc:
        import contextlib

        with contextlib.ExitStack() as ctx:
            consts = ctx.enter_context(tc.tile_pool(name="consts", bufs=1))
            qkvt = ctx.enter_context(tc.tile_pool(name="qkvt", bufs=1))

            ident_f = consts.tile([128, 128], F32)
            make_identity(nc, ident_f)
            ident = consts.tile([128, 128], F32R)
            nc.vector.tensor_copy(ident, ident_f)
            ones_bf = consts.tile([128, 128], BF16)
            nc.gpsimd.memset(ones_bf, 1.0)
            bq_sb = consts.tile([128, R], F32)
            bk_sb = consts.tile([128, 1], F32)
            bv_sb = consts.tile([128, 1], F32)

            QT = qkvt.tile([128, R, S], F32R)    # QT[d, h, s]
            KT = qkvt.tile([128, S], F32R)       # KT[d, t]
            V = qkvt.tile([128, ST, D], BF16)    # V[t%128, tt, d]

            # ---- phase 1: QKV^T projections + V transpose ----
            with tc.tile_pool(name="vt", bufs=1) as vtpool, \
                 tc.tile_pool(name="wts", bufs=1) as wpool, \
                 tc.tile_pool(name="xts", bufs=4) as xtpool, \
                 tc.tile_pool(name="ps1", bufs=7, space="PSUM") as ps1, \
                 tc.tile_pool(name="psv", bufs=1, space="PSUM") as psv:
                VT = vtpool.tile([128, S], F32R)
                wq_sb = wpool.tile([128, ET, R * D], F32R)
                wk_sb = wpool.tile([128, ET, D], F32R)
                wv_sb = wpool.tile([128, ET, D], F32R)
                # e-granular DMAs for the first quarter so the first
                # matmuls unblock asap; remaining quarters spread over the
                # SP and ACT DGE queues so neither queue serializes >3MB
                nc.sync.dma_start(wq_sb[:, 0:1], wqr[:, 0:1])
                nc.scalar.dma_start(wk_sb[:, 0:4], wkr[:, 0:4])
                nc.scalar.dma_start(wv_sb[:, 0:4], wvr[:, 0:4])
                nc.sync.dma_start(wq_sb[:, 1:4], wqr[:, 1:4])
                nc.sync.dma_start(wq_sb[:, 4:8], wqr[:, 4:8])
                for half in (slice(4, 10), slice(10, ET)):
                    nc.scalar.dma_start(wk_sb[:, half], wkr[:, half])
                    nc.scalar.dma_start(wv_sb[:, half], wvr[:, half])
                # tail wq quarters ride the scalar ring: the sync ring's
                # serial transfer rate can't deliver all 4MB of wq in time
                nc.scalar.dma_start(wq_sb[:, 8:12], wqr[:, 8:12])
                nc.scalar.dma_start(wq_sb[:, 12:16], wqr[:, 12:16])
                # biases are tiny and needed late; issue after the weights
                nc.sync.dma_start(bq_sb, bqv.rearrange("(o p) -> p o", p=128))
                nc.sync.dma_start(bk_sb, bkv.rearrange("(o p) -> p o", p=128))
                nc.sync.dma_start(bv_sb, bvv.rearrange("(o p) -> p o", p=128))

                def transposes(sc):
                    tps = psv.tile([128, 512], F32R, tag="pv", name="tps")
                    for i in range(4):
                        tt = sc * 4 + i
                        nc.tensor.transpose(
                            tps[:, i * 128 : (i + 1) * 128],
                            VT[:, tt * 128 : (tt + 1) * 128],
                            ident,
                        )
                    for i in range(4):
                        nc.vector.tensor_copy(
                            V[:, sc * 4 + i], tps[:, i * 128 : (i + 1) * 128]
                        )

                for sc in range(SC):
                    cs = slice(sc * 512, (sc + 1) * 512)
                    po = [ps1.tile([128, 512], F32, tag="p1", name="po")
                          for _ in range(R + 2)]
                    for eq in range(4):
                        xq = xtpool.tile([128, 4, 512], F32R, tag="xt")
                        if sc == 0 and eq == 0:
                            # e-granular so the first matmul starts early
                            for i in range(4):
                                nc.gpsimd.dma_start(
                                    xq[:, i : i + 1], xTr[:, i : i + 1, cs]
                                )
                        else:
                            nc.gpsimd.dma_start(
                                xq, xTr[:, eq * 4 : eq * 4 + 4, cs]
                            )
                        for i in range(4):
                            e = eq * 4 + i
                            for ot in range(R + 2):
                                if ot < R:
                                    lhsT = wq_sb[:, e, ot * 128 : (ot + 1) * 128]
                                elif ot == R:
                                    lhsT = wk_sb[:, e]
                                else:
                                    lhsT = wv_sb[:, e]
                                nc.tensor.matmul(
                                    po[ot], lhsT, xq[:, i],
                                    start=(e == 0), stop=(e == ET - 1),
                                )
                        if eq == 1 and sc > 0:
                            # previous chunk's V rows are long since
                            # drained; transpose them here so PE never
                            # waits on the ACT drain queue
                            transposes(sc - 1)
                    # drains; for the last chunk emit V first so its
                    # transposes (right below) wait minimally
                    drains = [(VT[:, cs], po[R + 1], bv_sb[:, 0:1]),
                              (KT[:, cs], po[R], bk_sb[:, 0:1])]
                    qdr = [(QT[:, ot, cs], po[ot], bq_sb[:, ot : ot + 1])
                           for ot in range(R)]
                    order = drains + qdr if sc == SC - 1 else qdr + drains[::-1]
                    for dst, src, bias in order:
                        nc.scalar.add(dst, src, bias)
                transposes(SC - 1)

            # ---- phase 2: attention, software-pipelined ----
            p23 = ctx.enter_context(tc.tile_pool(name="p23", bufs=1))
            outT = p23.tile([128, R, S], F32R)  # normalized attn outT[d, h, s]
            wo_sb = p23.tile([128, R, E], F32R)
            for q in range(4):
                nc.sync.dma_start(wo_sb[:, q], wor[:, q])

            with tc.tile_pool(name="ps_av", bufs=1, space="PSUM") as ps_av, \
                 tc.tile_pool(name="probs", bufs=10) as probs_pool, \
                 tc.tile_pool(name="accp", bufs=2) as accp, \
                 tc.tile_pool(name="avsb", bufs=2) as avsb, \
                 tc.tile_pool(name="smsb", bufs=2) as smsb, \
                 tc.tile_pool(name="rcp", bufs=2) as rcp:

                pss_t = {}   # j -> scores psum tile
                acc_t = {}   # blk -> (dve_acc, pool_acc) bf16 accumulators
                av_t = {}    # blk -> AV psum tile
                avsb_t = {}  # blk -> AV sbuf drain tile
                pt_tail = {}  # blk -> tail prob tiles summed directly on PE

                def finalize(b, sums_pool, sums_tag):
                    """Deferred per-block tail: reduce the prob accumulators
                    (+ the tail prob tiles, summed directly on PE to offload
                    DVE/Pool), reciprocal, and normalize into outT.  Runs 5
                    tiles into the next block so nothing on PE ever waits
                    for it.  The sums PSUM tile is drained to SBUF by a fast
                    DVE copy before the slow reciprocal reads it, so the
                    PSUM slot recycles in ~0.7us instead of ~6.5us (the
                    reciprocal-blocks-scores WAR stall)."""
                    h, pr = b // NPAIR, b % NPAIR
                    q0 = pr * 1024
                    acc_d, acc_p = acc_t.pop(b)
                    srcs = [acc_d, acc_p] + pt_tail.pop(b)
                    sums = sums_pool.tile(
                        [128, 1024], F32, tag=sums_tag, name="sums"
                    )
                    for hf in range(2):
                        hs = slice(hf * 512, (hf + 1) * 512)
                        for si, src in enumerate(srcs):
                            nc.tensor.matmul(
                                sums[:, hs], ones_bf, src[:, hs],
                                start=(si == 0), stop=(si == len(srcs) - 1),
                            )
                    ssb = smsb.tile([128, 1024], F32, tag="ssb", name="ssb")
                    nc.vector.tensor_copy(ssb, sums)
                    rc = rcp.tile([128, 1024], F32, tag="rc", name="rc")
                    nc.vector.reciprocal(rc, ssb)
                    # normalize on DVE right after the reciprocal so phase 3
                    # never waits behind the next block's Pool adds
                    nc.vector.tensor_tensor(
                        outT[:, h, q0 : q0 + 1024], avsb_t.pop(b), rc, Mult
                    )

                with tc.tile_pool(name="ps_mix", bufs=3, space="PSUM") as ps_mix:
                    for j in range(16 * NBLK + LOOK):
                        if j < 16 * NBLK:
                            b, tt = j // ST, j % ST
                            h, pr = b // NPAIR, b % NPAIR
                            q0 = pr * 1024
                            pss = ps_mix.tile(
                                [128, 1024], F32, tag="s", name="pss"
                            )
                            kslice = KT[:, tt * 128 : (tt + 1) * 128]
                            for hf in range(2):
                                nc.tensor.matmul(
                                    pss[:, hf * 512 : (hf + 1) * 512],
                                    kslice,
                                    QT[:, h, q0 + hf * 512 : q0 + (hf + 1) * 512],
                                    start=True, stop=True,
                                )
                            pss_t[j] = pss
                        jj = j - LOOK
                        if 0 <= jj < 16 * NBLK:
                            b, tt = jj // ST, jj % ST
                            h, pr = b // NPAIR, b % NPAIR
                            q0 = pr * 1024
                            pt = probs_pool.tile(
                                [128, 1024], BF16, tag="pt", name="pt"
                            )
                            nc.scalar.activation(pt, pss_t.pop(jj), Exp)
                            # denominator accumulation split, sized from
                            # measured per-op costs so each engine stays
                            # under PE's per-block time: DVE gets the init
                            # copy + 4 even tiles, Pool 6 tiles, PE the last
                            # 5 via finalize's ones-matmul
                            if tt == 0:
                                acc_d = accp.tile(
                                    [128, 1024], BF16, tag="accd", name="accd"
                                )
                                acc_p = accp.tile(
                                    [128, 1024], BF16, tag="accp", name="accp"
                                )
                                acc_t[b] = (acc_d, acc_p)
                                pt_tail[b] = []
                                nc.vector.tensor_copy(acc_d, pt)
                                nc.gpsimd.memset(acc_p, 0.0)
                                av_t[b] = ps_av.tile(
                                    [128, 1024], F32, tag="av", name="avp"
                                )
                            elif tt >= (ST - 7 if b == NBLK - 1 else ST - 5):
                                # last block hands two extra tiles to PE:
                                # there is no following block to hide the
                                # Pool adds' latency behind
                                pt_tail[b].append(pt)
                            elif tt % 2 == 0 and tt <= 8:
                                acc_d = acc_t[b][0]
                                nc.vector.tensor_tensor(acc_d, acc_d, pt, Add)
                            else:
                                acc_p = acc_t[b][1]
                                nc.gpsimd.tensor_tensor(acc_p, acc_p, pt, Add)
                            av = av_t[b]
                            for hf in range(2):
                                hs = slice(hf * 512, (hf + 1) * 512)
                                nc.tensor.matmul(
                                    av[:, hs], V[:, tt], pt[:, hs],
                                    start=(tt == 0), stop=(tt == ST - 1),
                                )
                            if tt == ST - 1:
                                # drain AV psum immediately so the next
                                # block's AV start never waits on the
                                # normalize chain
                                asb = avsb.tile(
                                    [128, 1024], F32, tag="avsb", name="asb"
                                )
                                avsb_t[b] = asb
                                nc.vector.tensor_copy(asb, av_t.pop(b))
                        # per-block tail emitted after this iteration's
                        # scores so the exp pipeline is never starved by
                        # the 14-matmul sums burst
                        if j >= 21 and (j - 21) % 16 == 0 and (j - 21) // 16 < NBLK - 1:
                            finalize((j - 21) // 16, ps_mix, "s")

                    # last block's reduction goes through the AV pool (not
                    # ps_mix), emitted before ps_mix closes so the close and
                    # phase 3's pool open overlap the final reciprocal
                    finalize(NBLK - 1, ps_av, "av")

                # ---- phase 3: output projection (transposed) ----
                otr = otd.rearrange("(o p) m -> p o m", p=128)
                with tc.tile_pool(name="ostage", bufs=4) as ostage, \
                     tc.tile_pool(name="ps_o", bufs=6, space="PSUM") as ps_o:
                    # sc outer: the first tiles only need the pr=0 (even)
                    # blocks, so P3 never waits on the last blocks' deferred
                    # normalize; output DMAs go out in et-pairs alternating
                    # between two DGE rings so the ring never backs up
                    for sc in range(SC):
                        cs = slice(sc * 512, (sc + 1) * 512)
                        for etp in range(ET // 2):
                            st = ostage.tile(
                                [128, 2, 512], F32, tag="ost", name="st"
                            )
                            for k in range(2):
                                et = etp * 2 + k
                                ps = ps_o.tile(
                                    [128, 512], F32, tag="po", name="ps"
                                )
                                for h in range(R):
                                    nc.tensor.matmul(
                                        ps,
                                        wo_sb[:, h, et * 128 : (et + 1) * 128],
                                        outT[:, h, cs],
                                        start=(h == 0), stop=(h == R - 1),
                                    )
                                nc.vector.tensor_copy(st[:, k], ps)
                            eng = (nc.sync, nc.scalar, nc.gpsimd)[etp % 3]
                            eng.dma_start(
                                otr[:, etp * 2 : etp * 2 + 2, cs], st
                            )

    _split_multi_waits(nc)
    return nc


def _prepare(x, Wq, bq, Wk, bk, Wv, bv, Wo, bo):
    """Host-side sharding: build per-core input maps."""
    x = np.asarray(x, dtype=np.float32)
    Wq = np.asarray(Wq, dtype=np.float32)
    bq = np.asarray(bq, dtype=np.float32)
    Wk = np.asarray(Wk, dtype=np.float32)
    bk = np.asarray(bk, dtype=np.float32)
    Wv = np.asarray(Wv, dtype=np.float32)
    bv = np.asarray(bv, dtype=np.float32)
    Wo = np.asarray(Wo, dtype=np.float32)

    isd = np.float32(1.0 / np.sqrt(D))
    xTs = [np.ascontiguousarray(x[b].T) for b in range(B)]
    in_maps = []
    for core in range(8):
        b, g = divmod(core, G)
        in_maps.append({
            "xT": xTs[b],
            "wq": np.ascontiguousarray(Wq[:, g * R * D : (g + 1) * R * D]) * isd,
            "wk": np.ascontiguousarray(Wk[:, g * D : (g + 1) * D]),
            "wv": np.ascontiguousarray(Wv[:, g * D : (g + 1) * D]),
            "wo": np.ascontiguousarray(Wo[g * R * D : (g + 1) * R * D, :]),
            "bqv": bq[g * R * D : (g + 1) * R * D] * isd,
            "bkv": bk[g * D : (g + 1) * D],
            "bvv": bv[g * D : (g + 1) * D],
        })
    return in_maps


def _gather(results, bo):
    bo = np.asarray(bo, dtype=np.float32)
    out = np.empty((B, S, E), dtype=np.float32)
    for b in range(B):
        acc = results[b * G]["ot"].copy()
        for g in range(1, G):
            acc += results[b * G + g]["ot"]
        out[b] = acc.T + bo
    return out


def kernel(x, Wq, bq, Wk, bk, Wv, bv, Wo, bo):
    from concourse.bass_utils import run_bass_kernel_spmd

    if "nc" not in _cache:
        _cache["nc"] = _build_program()
    nc = _cache["nc"]
    in_maps = _prepare(x, Wq, bq, Wk, bk, Wv, bv, Wo, bo)
    res = run_bass_kernel_spmd(nc, in_maps, core_ids=list(range(8)))
    return _gather(res.results, bo)


# revision 35
# speedup vs baseline: 1.5281x; 1.0473x over previous
"""GQA attention kernel for 8 Trainium2 NeuronCores.

Sharding: core = (batch b, kv_group g), b in {0,1}, g in {0..3}.
Each core computes the 4 heads of one KV group for one batch and the
partial output projection for those heads; the host sums the 4 group
partials per batch.  Zero duplicated compute across cores.

v4 design (baseline was 516us):
  - P1 (QKV proj) e-outer with 6 concurrent PSUM accumulation groups;
    e-granular first-quarter DMAs spread across the SP/ACT/Pool DGE
    queues so the first matmul starts ~12us in and PE never waits on
    DMA; V transposed through a side PSUM bank inside the loop.
  - P2 (attention) software-pipelined with lookahead-2 scores in a
    3-deep PSUM rotation so PE never stalls on the ACT exp chain.
  - softmax denominators: probs written bf16; accumulation split
    across DVE (5 tiles + init copy), Pool (7 tiles), and PE
    (3 tail tiles via the final ones-matmul), sized from measured
    per-op costs so every engine stays under PE's per-block time.
  - per-block normalize (reciprocal + multiply) deferred 4 tiles into
    the next block, AV PSUM drained immediately by DVE, so no PE
    instruction ever waits on the normalize chain; the last block's
    reduction uses the AV PSUM pool so phase 3's PSUM pool opens
    without waiting on it.
  - numerics: scores/Q/K/weights stay fp32r; only probs/V/acc are
    bf16 (validated 2.1e-3 max rel err vs 2e-2 budget).
"""

import numpy as np

# problem shape (hardcoded per contract)
B, S, E = 2, 2048, 2048
H, G, D = 16, 4, 128
R = H // G          # heads per kv group = 4
KV = G * D          # 512
ST = S // 128       # 16 t-tiles
ET = E // 128       # 16 e-tiles
SC = S // 512       # 4 s-chunks
NPAIR = S // 1024   # 2 q-chunk pairs
NBLK = R * NPAIR    # 8 attention blocks per core
LOOK = 2            # scores lookahead (PSUM rotation depth - 1)

_cache = {}


def _split_multi_waits(nc, maxw=1):
    """Walrus in this container accepts only one sync-wait per
    instruction; move extra waits onto preceding same-engine NoOps."""
    from concourse import mybir

    n_split = 0
    for fn in nc.m.functions:
        for bb in fn.blocks:
            out = []
            changed = False
            for inst in bb.instructions:
                si = inst.sync_info
                waits = list(si.on_wait or []) if si is not None else []
                if len(waits) > maxw:
                    changed = True
                    n_split += 1
                    head, tail = waits[:-maxw], waits[-maxw:]
                    for j in range(0, len(head), maxw):
                        nop = mybir.InstNoOp(
                            name=f"{inst.name}-wsplit{j}", ins=[], outs=[]
                        )
                        nop.engine = inst.engine
                        nop.sync_info = mybir.SyncInfo(
                            on_wait=head[j : j + maxw], on_update=[]
                        )
                        out.append(nop)
                    si.on_wait = tail
                out.append(inst)
            if changed:
                bb.instructions = out
    return n_split


def _build_program():
    import concourse.bass as bass
    import concourse.tile as tile
    from concourse import mybir
    from concourse.masks import make_identity

    F32R = mybir.dt.float32r
    F32 = mybir.dt.float32
    BF16 = mybir.dt.bfloat16
    Exp = mybir.ActivationFunctionType.Exp
    Mult = mybir.AluOpType.mult
    Add = mybir.AluOpType.add

    nc = bass.Bass(target_bir_lowering=False)

    # x and the QKV weights arrive as bf16 (host-cast): halves input DMA
    # bytes — the DGE rings are the phase-1 constraint — at identical
    # matmul throughput (1 row/cycle for bf16 and fp32r alike)
    xT = nc.dram_tensor("xT", [E, S], BF16, kind="ExternalInput")
    wq = nc.dram_tensor("wq", [E, R * D], BF16, kind="ExternalInput")
    wk = nc.dram_tensor("wk", [E, D], BF16, kind="ExternalInput")
    wv = nc.dram_tensor("wv", [E, D], BF16, kind="ExternalInput")
    wo = nc.dram_tensor("wo", [R * D, E], F32R, kind="ExternalInput")
    bqv = nc.dram_tensor("bqv", [R * D], F32, kind="ExternalInput")
    bkv = nc.dram_tensor("bkv", [D], F32, kind="ExternalInput")
    bvv = nc.dram_tensor("bvv", [D], F32, kind="ExternalInput")
    otd = nc.dram_tensor("ot", [E, S], F32, kind="ExternalOutput")

    xTr = xT.rearrange("(o p) m -> p o m", p=128)
    wqr = wq.rearrange("(o p) m -> p o m", p=128)
    wkr = wk.rearrange("(o p) m -> p o m", p=128)
    wvr = wv.rearrange("(o p) m -> p o m", p=128)
    wor = wo.rearrange("(o p) m -> p o m", p=128)

    with tile.TileContext(nc) as tc:
        import contextlib

        with contextlib.ExitStack() as ctx:
            consts = ctx.enter_context(tc.tile_pool(name="consts", bufs=1))
            qkvt = ctx.enter_context(tc.tile_pool(name="qkvt", bufs=1))

            ident_f = consts.tile([128, 128], F32)
            make_identity(nc, ident_f)
            ident = consts.tile([128, 128], F32R)
            nc.vector.tensor_copy(ident, ident_f)
            ones_bf = consts.tile([128, 128], BF16)
            nc.gpsimd.memset(ones_bf, 1.0)
            bq_sb = consts.tile([128, R], F32)
            bk_sb = consts.tile([128, 1], F32)
            bv_sb = consts.tile([128, 1], F32)

            QT = qkvt.tile([128, R, S], F32R)    # QT[d, h, s]
            KT = qkvt.tile([128, S], F32R)       # KT[d, t]
            V = qkvt.tile([128, ST, D], BF16)    # V[t%128, tt, d]

            # ---- phase 1: QKV^T projections + V transpose ----
            with tc.tile_pool(name="vt", bufs=1) as vtpool, \
                 tc.tile_pool(name="wts", bufs=1) as wpool, \
                 tc.tile_pool(name="xts", bufs=4) as xtpool, \
                 tc.tile_pool(name="ps1", bufs=7, space="PSUM") as ps1, \
                 tc.tile_pool(name="psv", bufs=1, space="PSUM") as psv:
                VT = vtpool.tile([128, S], F32R)
                wq_sb = wpool.tile([128, ET, R * D], BF16)
                wk_sb = wpool.tile([128, ET, D], BF16)
                wv_sb = wpool.tile([128, ET, D], BF16)
                # e-granular DMAs for the first quarter so the first
                # matmuls unblock asap; remaining quarters spread over the
                # SP and ACT DGE queues so neither queue serializes >3MB
                nc.sync.dma_start(wq_sb[:, 0:1], wqr[:, 0:1])
                nc.scalar.dma_start(wk_sb[:, 0:4], wkr[:, 0:4])
                nc.scalar.dma_start(wv_sb[:, 0:4], wvr[:, 0:4])
                nc.sync.dma_start(wq_sb[:, 1:4], wqr[:, 1:4])
                for q in range(1, 4):
                    sl = slice(q * 4, q * 4 + 4)
                    nc.sync.dma_start(wq_sb[:, sl], wqr[:, sl])
                for half in (slice(4, 10), slice(10, ET)):
                    nc.scalar.dma_start(wk_sb[:, half], wkr[:, half])
                    nc.scalar.dma_start(wv_sb[:, half], wvr[:, half])
                # biases are tiny and needed late; issue after the weights
                nc.sync.dma_start(bq_sb, bqv.rearrange("(o p) -> p o", p=128))
                nc.sync.dma_start(bk_sb, bkv.rearrange("(o p) -> p o", p=128))
                nc.sync.dma_start(bv_sb, bvv.rearrange("(o p) -> p o", p=128))

                def transposes(sc):
                    tps = psv.tile([128, 512], F32R, tag="pv", name="tps")
                    for i in range(4):
                        tt = sc * 4 + i
                        nc.tensor.transpose(
                            tps[:, i * 128 : (i + 1) * 128],
                            VT[:, tt * 128 : (tt + 1) * 128],
                            ident,
                        )
                    for i in range(4):
                        nc.vector.tensor_copy(
                            V[:, sc * 4 + i], tps[:, i * 128 : (i + 1) * 128]
                        )

                for sc in range(SC):
                    cs = slice(sc * 512, (sc + 1) * 512)
                    po = [ps1.tile([128, 512], F32, tag="p1", name="po")
                          for _ in range(R + 2)]
                    for eq in range(4):
                        xq = xtpool.tile([128, 4, 512], BF16, tag="xt")
                        if sc == 0 and eq == 0:
                            # e-granular so the first matmul starts early
                            for i in range(4):
                                nc.gpsimd.dma_start(
                                    xq[:, i : i + 1], xTr[:, i : i + 1, cs]
                                )
                        else:
                            nc.gpsimd.dma_start(
                                xq, xTr[:, eq * 4 : eq * 4 + 4, cs]
                            )
                        for i in range(4):
                            e = eq * 4 + i
                            for ot in range(R + 2):
                                if ot < R:
                                    lhsT = wq_sb[:, e, ot * 128 : (ot + 1) * 128]
                                elif ot == R:
                                    lhsT = wk_sb[:, e]
                                else:
                                    lhsT = wv_sb[:, e]
                                nc.tensor.matmul(
                                    po[ot], lhsT, xq[:, i],
                                    start=(e == 0), stop=(e == ET - 1),
                                )
                        if eq == 1 and sc > 0:
                            # previous chunk's V rows are long since
                            # drained; transpose them here so PE never
                            # waits on the ACT drain queue
                            transposes(sc - 1)
                    # drains; for the last chunk emit V first so its
                    # transposes (right below) wait minimally
                    drains = [(VT[:, cs], po[R + 1], bv_sb[:, 0:1]),
                              (KT[:, cs], po[R], bk_sb[:, 0:1])]
                    qdr = [(QT[:, ot, cs], po[ot], bq_sb[:, ot : ot + 1])
                           for ot in range(R)]
                    order = drains + qdr if sc == SC - 1 else qdr + drains[::-1]
                    for dst, src, bias in order:
                        nc.scalar.add(dst, src, bias)
                transposes(SC - 1)

            # ---- phase 2: attention, software-pipelined ----
            p23 = ctx.enter_context(tc.tile_pool(name="p23", bufs=1))
            outT = p23.tile([128, R, S], F32R)  # normalized attn outT[d, h, s]
            wo_sb = p23.tile([128, R, E], F32R)
            for q in range(4):
                nc.sync.dma_start(wo_sb[:, q], wor[:, q])

            with tc.tile_pool(name="ps_av", bufs=1, space="PSUM") as ps_av, \
                 tc.tile_pool(name="probs", bufs=10) as probs_pool, \
                 tc.tile_pool(name="accp", bufs=2) as accp, \
                 tc.tile_pool(name="avsb", bufs=2) as avsb, \
                 tc.tile_pool(name="smsb", bufs=2) as smsb, \
                 tc.tile_pool(name="rcp", bufs=2) as rcp:

                pss_t = {}   # j -> scores psum tile
                acc_t = {}   # blk -> (dve_acc, pool_acc) bf16 accumulators
                av_t = {}    # blk -> AV psum tile
                avsb_t = {}  # blk -> AV sbuf drain tile
                pt_tail = {}  # blk -> tail prob tiles summed directly on PE

                def finalize(b, sums_pool, sums_tag):
                    """Deferred per-block tail: reduce the prob accumulators
                    (+ the tail prob tiles, summed directly on PE to offload
                    DVE/Pool), reciprocal, and normalize into outT.  Runs 5
                    tiles into the next block so nothing on PE ever waits
                    for it.  The sums PSUM tile is drained to SBUF by a fast
                    DVE copy before the slow reciprocal reads it, so the
                    PSUM slot recycles in ~0.7us instead of ~6.5us (the
                    reciprocal-blocks-scores WAR stall)."""
                    h, pr = b // NPAIR, b % NPAIR
                    q0 = pr * 1024
                    acc_d, acc_p = acc_t.pop(b)
                    srcs = [acc_d, acc_p] + pt_tail.pop(b)
                    sums = sums_pool.tile(
                        [128, 1024], F32, tag=sums_tag, name="sums"
                    )
                    for hf in range(2):
                        hs = slice(hf * 512, (hf + 1) * 512)
                        for si, src in enumerate(srcs):
                            nc.tensor.matmul(
                                sums[:, hs], ones_bf, src[:, hs],
                                start=(si == 0), stop=(si == len(srcs) - 1),
                            )
                    ssb = smsb.tile([128, 1024], F32, tag="ssb", name="ssb")
                    nc.vector.tensor_copy(ssb, sums)
                    rc = rcp.tile([128, 1024], F32, tag="rc", name="rc")
                    nc.vector.reciprocal(rc, ssb)
                    # normalize on DVE right after the reciprocal so phase 3
                    # never waits behind the next block's Pool adds
                    nc.vector.tensor_tensor(
                        outT[:, h, q0 : q0 + 1024], avsb_t.pop(b), rc, Mult
                    )

                with tc.tile_pool(name="ps_mix", bufs=3, space="PSUM") as ps_mix:
                    for j in range(16 * NBLK + LOOK):
                        if j < 16 * NBLK:
                            b, tt = j // ST, j % ST
                            h, pr = b // NPAIR, b % NPAIR
                            q0 = pr * 1024
                            pss = ps_mix.tile(
                                [128, 1024], F32, tag="s", name="pss"
                            )
                            kslice = KT[:, tt * 128 : (tt + 1) * 128]
                            for hf in range(2):
                                nc.tensor.matmul(
                                    pss[:, hf * 512 : (hf + 1) * 512],
                                    kslice,
                                    QT[:, h, q0 + hf * 512 : q0 + (hf + 1) * 512],
                                    start=True, stop=True,
                                )
                            pss_t[j] = pss
                        jj = j - LOOK
                        if 0 <= jj < 16 * NBLK:
                            b, tt = jj // ST, jj % ST
                            h, pr = b // NPAIR, b % NPAIR
                            q0 = pr * 1024
                            pt = probs_pool.tile(
                                [128, 1024], BF16, tag="pt", name="pt"
                            )
                            nc.scalar.activation(pt, pss_t.pop(jj), Exp)
                            # denominator accumulation split, sized from
                            # measured per-op costs so each engine stays
                            # under PE's per-block time: DVE gets the init
                            # copy + 4 even tiles, Pool 6 tiles, PE the last
                            # 5 via finalize's ones-matmul
                            if tt == 0:
                                acc_d = accp.tile(
                                    [128, 1024], BF16, tag="accd", name="accd"
                                )
                                acc_p = accp.tile(
                                    [128, 1024], BF16, tag="accp", name="accp"
                                )
                                acc_t[b] = (acc_d, acc_p)
                                pt_tail[b] = []
                                nc.vector.tensor_copy(acc_d, pt)
                                nc.gpsimd.memset(acc_p, 0.0)
                                av_t[b] = ps_av.tile(
                                    [128, 1024], F32, tag="av", name="avp"
                                )
                            elif tt >= (ST - 7 if b == NBLK - 1 else ST - 5):
                                # last block hands two extra tiles to PE:
                                # there is no following block to hide the
                                # Pool adds' latency behind
                                pt_tail[b].append(pt)
                            elif tt % 2 == 0 and tt <= 8:
                                acc_d = acc_t[b][0]
                                nc.vector.tensor_tensor(acc_d, acc_d, pt, Add)
                            else:
                                acc_p = acc_t[b][1]
                                nc.gpsimd.tensor_tensor(acc_p, acc_p, pt, Add)
                            av = av_t[b]
                            for hf in range(2):
                                hs = slice(hf * 512, (hf + 1) * 512)
                                nc.tensor.matmul(
                                    av[:, hs], V[:, tt], pt[:, hs],
                                    start=(tt == 0), stop=(tt == ST - 1),
                                )
                            if tt == ST - 1:
                                # drain AV psum immediately so the next
                                # block's AV start never waits on the
                                # normalize chain
                                asb = avsb.tile(
                                    [128, 1024], F32, tag="avsb", name="asb"
                                )
                                avsb_t[b] = asb
                                nc.vector.tensor_copy(asb, av_t.pop(b))
                        # per-block tail emitted after this iteration's
                        # scores so the exp pipeline is never starved by
                        # the 14-matmul sums burst
                        if j >= 21 and (j - 21) % 16 == 0 and (j - 21) // 16 < NBLK - 1:
                            finalize((j - 21) // 16, ps_mix, "s")

                    # last block's reduction goes through the AV pool (not
                    # ps_mix), emitted before ps_mix closes so the close and
                    # phase 3's pool open overlap the final reciprocal
                    finalize(NBLK - 1, ps_av, "av")

                # ---- phase 3: output projection (transposed) ----
                otr = otd.rearrange("(o p) m -> p o m", p=128)
                with tc.tile_pool(name="ostage", bufs=4) as ostage, \
                     tc.tile_pool(name="ps_o", bufs=6, space="PSUM") as ps_o:
                    # sc outer: the first tiles only need the pr=0 (even)
                    # blocks, so P3 never waits on the last blocks' deferred
                    # normalize; output DMAs go out in et-pairs alternating
                    # between two DGE rings so the ring never backs up
                    for sc in range(SC):
                        cs = slice(sc * 512, (sc + 1) * 512)
                        for etp in range(ET // 2):
                            st = ostage.tile(
                                [128, 2, 512], F32, tag="ost", name="st"
                            )
                            for k in range(2):
                                et = etp * 2 + k
                                ps = ps_o.tile(
                                    [128, 512], F32, tag="po", name="ps"
                                )
                                for h in range(R):
                                    nc.tensor.matmul(
                                        ps,
                                        wo_sb[:, h, et * 128 : (et + 1) * 128],
                                        outT[:, h, cs],
                                        start=(h == 0), stop=(h == R - 1),
                                    )
                                nc.vector.tensor_copy(st[:, k], ps)
                            eng = (nc.sync, nc.scalar, nc.gpsimd)[etp % 3]
                            eng.dma_start(
                                otr[:, etp * 2 : etp * 2 + 2, cs], st
                            )

    _split_multi_waits(nc)
    return nc


def _prepare(x, Wq, bq, Wk, bk, Wv, bv, Wo, bo):
    """Host-side sharding: build per-core input maps."""
    x = np.asarray(x, dtype=np.float32)
    Wq = np.asarray(Wq, dtype=np.float32)
    bq = np.asarray(bq, dtype=np.float32)
    Wk = np.asarray(Wk, dtype=np.float32)
    bk = np.asarray(bk, dtype=np.float32)
    Wv = np.asarray(Wv, dtype=np.float32)
    bv = np.asarray(bv, dtype=np.float32)
    Wo = np.asarray(Wo, dtype=np.float32)

    import ml_dtypes

    BF = ml_dtypes.bfloat16
    isd = np.float32(1.0 / np.sqrt(D))
    xTs = [np.ascontiguousarray(x[b].T.astype(BF)) for b in range(B)]
    in_maps = []
    for core in range(8):
        b, g = divmod(core, G)
        in_maps.append({
            "xT": xTs[b],
            "wq": np.ascontiguousarray(
                (Wq[:, g * R * D : (g + 1) * R * D] * isd).astype(BF)
            ),
            "wk": np.ascontiguousarray(Wk[:, g * D : (g + 1) * D].astype(BF)),
            "wv": np.ascontiguousarray(Wv[:, g * D : (g + 1) * D].astype(BF)),
            "wo": np.ascontiguousarray(Wo[g * R * D : (g + 1) * R * D, :]),
            "bqv": bq[g * R * D : (g + 1) * R * D] * isd,
            "bkv": bk[g * D : (g + 1) * D],
            "bvv": bv[g * D : (g + 1) * D],
        })
    return in_maps


def _gather(results, bo):
    bo = np.asarray(bo, dtype=np.float32)
    out = np.empty((B, S, E), dtype=np.float32)
    for b in range(B):
        acc = results[b * G]["ot"].copy()
        for g in range(1, G):
            acc += results[b * G + g]["ot"]
        out[b] = acc.T + bo
    return out


def kernel(x, Wq, bq, Wk, bk, Wv, bv, Wo, bo):
    from concourse.bass_utils import run_bass_kernel_spmd

    if "nc" not in _cache:
        _cache["nc"] = _build_program()
    nc = _cache["nc"]
    in_maps = _prepare(x, Wq, bq, Wk, bk, Wv, bv, Wo, bo)
    res = run_bass_kernel_spmd(nc, in_maps, core_ids=list(range(8)))
    return _gather(res.results, bo)


# revision 37
# speedup vs baseline: 1.5399x; 1.0078x over previous
"""GQA attention kernel for 8 Trainium2 NeuronCores.

Sharding: core = (batch b, kv_group g), b in {0,1}, g in {0..3}.
Each core computes the 4 heads of one KV group for one batch and the
partial output projection for those heads; the host sums the 4 group
partials per batch.  Zero duplicated compute across cores.

v4 design (baseline was 516us):
  - P1 (QKV proj) e-outer with 6 concurrent PSUM accumulation groups;
    e-granular first-quarter DMAs spread across the SP/ACT/Pool DGE
    queues so the first matmul starts ~12us in and PE never waits on
    DMA; V transposed through a side PSUM bank inside the loop.
  - P2 (attention) software-pipelined with lookahead-2 scores in a
    3-deep PSUM rotation so PE never stalls on the ACT exp chain.
  - softmax denominators: probs written bf16; accumulation split
    across DVE (5 tiles + init copy), Pool (7 tiles), and PE
    (3 tail tiles via the final ones-matmul), sized from measured
    per-op costs so every engine stays under PE's per-block time.
  - per-block normalize (reciprocal + multiply) deferred 4 tiles into
    the next block, AV PSUM drained immediately by DVE, so no PE
    instruction ever waits on the normalize chain; the last block's
    reduction uses the AV PSUM pool so phase 3's PSUM pool opens
    without waiting on it.
  - numerics: scores/Q/K/weights stay fp32r; only probs/V/acc are
    bf16 (validated 2.1e-3 max rel err vs 2e-2 budget).
"""

import numpy as np

# problem shape (hardcoded per contract)
B, S, E = 2, 2048, 2048
H, G, D = 16, 4, 128
R = H // G          # heads per kv group = 4
KV = G * D          # 512
ST = S // 128       # 16 t-tiles
ET = E // 128       # 16 e-tiles
SC = S // 512       # 4 s-chunks
NPAIR = S // 1024   # 2 q-chunk pairs
NBLK = R * NPAIR    # 8 attention blocks per core
LOOK = 2            # scores lookahead (PSUM rotation depth - 1)

_cache = {}


def _split_multi_waits(nc, maxw=1):
    """Walrus in this container accepts only one sync-wait per
    instruction; move extra waits onto preceding same-engine NoOps."""
    from concourse import mybir

    n_split = 0
    for fn in nc.m.functions:
        for bb in fn.blocks:
            out = []
            changed = False
            for inst in bb.instructions:
                si = inst.sync_info
                waits = list(si.on_wait or []) if si is not None else []
                if len(waits) > maxw:
                    changed = True
                    n_split += 1
                    head, tail = waits[:-maxw], waits[-maxw:]
                    for j in range(0, len(head), maxw):
                        nop = mybir.InstNoOp(
                            name=f"{inst.name}-wsplit{j}", ins=[], outs=[]
                        )
                        nop.engine = inst.engine
                        nop.sync_info = mybir.SyncInfo(
                            on_wait=head[j : j + maxw], on_update=[]
                        )
                        out.append(nop)
                    si.on_wait = tail
                out.append(inst)
            if changed:
                bb.instructions = out
    return n_split


def _build_program():
    import concourse.bass as bass
    import concourse.tile as tile
    from concourse import mybir
    from concourse.masks import make_identity

    F32R = mybir.dt.float32r
    F32 = mybir.dt.float32
    BF16 = mybir.dt.bfloat16
    Exp = mybir.ActivationFunctionType.Exp
    Mult = mybir.AluOpType.mult
    Add = mybir.AluOpType.add

    nc = bass.Bass(target_bir_lowering=False)

    # x and the QKV weights arrive as bf16 (host-cast): halves input DMA
    # bytes — the DGE rings are the phase-1 constraint — at identical
    # matmul throughput (1 row/cycle for bf16 and fp32r alike)
    xT = nc.dram_tensor("xT", [E, S], BF16, kind="ExternalInput")
    wq = nc.dram_tensor("wq", [E, R * D], BF16, kind="ExternalInput")
    wk = nc.dram_tensor("wk", [E, D], BF16, kind="ExternalInput")
    wv = nc.dram_tensor("wv", [E, D], BF16, kind="ExternalInput")
    wo = nc.dram_tensor("wo", [R * D, E], F32R, kind="ExternalInput")
    bqv = nc.dram_tensor("bqv", [R * D], F32, kind="ExternalInput")
    bkv = nc.dram_tensor("bkv", [D], F32, kind="ExternalInput")
    bvv = nc.dram_tensor("bvv", [D], F32, kind="ExternalInput")
    otd = nc.dram_tensor("ot", [E, S], F32, kind="ExternalOutput")

    xTr = xT.rearrange("(o p) m -> p o m", p=128)
    wqr = wq.rearrange("(o p) m -> p o m", p=128)
    wkr = wk.rearrange("(o p) m -> p o m", p=128)
    wvr = wv.rearrange("(o p) m -> p o m", p=128)
    wor = wo.rearrange("(o p) m -> p o m", p=128)

    with tile.TileContext(nc) as tc:
        import contextlib

        with contextlib.ExitStack() as ctx:
            consts = ctx.enter_context(tc.tile_pool(name="consts", bufs=1))
            qkvt = ctx.enter_context(tc.tile_pool(name="qkvt", bufs=1))

            ident_f = consts.tile([128, 128], F32)
            make_identity(nc, ident_f)
            ident = consts.tile([128, 128], F32R)
            nc.vector.tensor_copy(ident, ident_f)
            ones_bf = consts.tile([128, 128], BF16)
            nc.gpsimd.memset(ones_bf, 1.0)
            bq_sb = consts.tile([128, R], F32)
            bk_sb = consts.tile([128, 1], F32)
            bv_sb = consts.tile([128, 1], F32)

            QT = qkvt.tile([128, R, S], F32R)    # QT[d, h, s]
            KT = qkvt.tile([128, S], F32R)       # KT[d, t]
            V = qkvt.tile([128, ST, D], BF16)    # V[t%128, tt, d]

            # ---- phase 1: QKV^T projections + V transpose ----
            with tc.tile_pool(name="vt", bufs=1) as vtpool, \
                 tc.tile_pool(name="wts", bufs=1) as wpool, \
                 tc.tile_pool(name="xts", bufs=4) as xtpool, \
                 tc.tile_pool(name="ps1", bufs=7, space="PSUM") as ps1, \
                 tc.tile_pool(name="psv", bufs=1, space="PSUM") as psv:
                VT = vtpool.tile([128, S], F32R)
                wq_sb = wpool.tile([128, ET, R * D], BF16)
                wk_sb = wpool.tile([128, ET, D], BF16)
                wv_sb = wpool.tile([128, ET, D], BF16)
                # e-granular DMAs for the first quarter so the first
                # matmuls unblock asap; remaining quarters spread over the
                # SP and ACT DGE queues so neither queue serializes >3MB
                nc.sync.dma_start(wq_sb[:, 0:1], wqr[:, 0:1])
                nc.scalar.dma_start(wk_sb[:, 0:4], wkr[:, 0:4])
                nc.scalar.dma_start(wv_sb[:, 0:4], wvr[:, 0:4])
                nc.sync.dma_start(wq_sb[:, 1:4], wqr[:, 1:4])
                nc.sync.dma_start(wq_sb[:, 4:8], wqr[:, 4:8])
                nc.scalar.dma_start(wq_sb[:, 8:12], wqr[:, 8:12])
                nc.scalar.dma_start(wq_sb[:, 12:16], wqr[:, 12:16])
                for half in (slice(4, 10), slice(10, ET)):
                    nc.sync.dma_start(wk_sb[:, half], wkr[:, half])
                    nc.sync.dma_start(wv_sb[:, half], wvr[:, half])
                # biases are tiny and needed late; issue after the weights
                nc.sync.dma_start(bq_sb, bqv.rearrange("(o p) -> p o", p=128))
                nc.sync.dma_start(bk_sb, bkv.rearrange("(o p) -> p o", p=128))
                nc.sync.dma_start(bv_sb, bvv.rearrange("(o p) -> p o", p=128))

                def transposes(sc):
                    tps = psv.tile([128, 512], F32R, tag="pv", name="tps")
                    for i in range(4):
                        tt = sc * 4 + i
                        nc.tensor.transpose(
                            tps[:, i * 128 : (i + 1) * 128],
                            VT[:, tt * 128 : (tt + 1) * 128],
                            ident,
                        )
                    for i in range(4):
                        nc.vector.tensor_copy(
                            V[:, sc * 4 + i], tps[:, i * 128 : (i + 1) * 128]
                        )

                for sc in range(SC):
                    cs = slice(sc * 512, (sc + 1) * 512)
                    po = [ps1.tile([128, 512], F32, tag="p1", name="po")
                          for _ in range(R + 2)]
                    for eq in range(4):
                        xq = xtpool.tile([128, 4, 512], BF16, tag="xt")
                        if sc == 0 and eq == 0:
                            # e-granular so the first matmul starts early
                            for i in range(4):
                                nc.gpsimd.dma_start(
                                    xq[:, i : i + 1], xTr[:, i : i + 1, cs]
                                )
                        elif sc == 0:
                            # pair-granular through the rest of the first
                            # chunk: the ring can't stay ahead of PE with
                            # full quarters this early
                            for i in (0, 2):
                                e0 = eq * 4 + i
                                nc.gpsimd.dma_start(
                                    xq[:, i : i + 2], xTr[:, e0 : e0 + 2, cs]
                                )
                        else:
                            nc.gpsimd.dma_start(
                                xq, xTr[:, eq * 4 : eq * 4 + 4, cs]
                            )
                        for i in range(4):
                            e = eq * 4 + i
                            for ot in range(R + 2):
                                if ot < R:
                                    lhsT = wq_sb[:, e, ot * 128 : (ot + 1) * 128]
                                elif ot == R:
                                    lhsT = wk_sb[:, e]
                                else:
                                    lhsT = wv_sb[:, e]
                                nc.tensor.matmul(
                                    po[ot], lhsT, xq[:, i],
                                    start=(e == 0), stop=(e == ET - 1),
                                )
                        if eq == 1 and sc > 0:
                            # previous chunk's V rows are long since
                            # drained; transpose them here so PE never
                            # waits on the ACT drain queue
                            transposes(sc - 1)
                    # drains; for the last chunk emit V first so its
                    # transposes (right below) wait minimally
                    drains = [(VT[:, cs], po[R + 1], bv_sb[:, 0:1]),
                              (KT[:, cs], po[R], bk_sb[:, 0:1])]
                    qdr = [(QT[:, ot, cs], po[ot], bq_sb[:, ot : ot + 1])
                           for ot in range(R)]
                    order = drains + qdr if sc == SC - 1 else qdr + drains[::-1]
                    for dst, src, bias in order:
                        nc.scalar.add(dst, src, bias)
                transposes(SC - 1)

            # ---- phase 2: attention, software-pipelined ----
            p23 = ctx.enter_context(tc.tile_pool(name="p23", bufs=1))
            outT = p23.tile([128, R, S], F32R)  # normalized attn outT[d, h, s]
            wo_sb = p23.tile([128, R, E], F32R)
            for q in range(4):
                nc.sync.dma_start(wo_sb[:, q], wor[:, q])

            with tc.tile_pool(name="ps_av", bufs=1, space="PSUM") as ps_av, \
                 tc.tile_pool(name="probs", bufs=10) as probs_pool, \
                 tc.tile_pool(name="accp", bufs=2) as accp, \
                 tc.tile_pool(name="avsb", bufs=2) as avsb, \
                 tc.tile_pool(name="smsb", bufs=2) as smsb, \
                 tc.tile_pool(name="rcp", bufs=2) as rcp:

                pss_t = {}   # j -> scores psum tile
                acc_t = {}   # blk -> (dve_acc, pool_acc) bf16 accumulators
                av_t = {}    # blk -> AV psum tile
                avsb_t = {}  # blk -> AV sbuf drain tile
                pt_tail = {}  # blk -> tail prob tiles summed directly on PE

                def finalize(b, sums_pool, sums_tag):
                    """Deferred per-block tail: reduce the prob accumulators
                    (+ the tail prob tiles, summed directly on PE to offload
                    DVE/Pool), reciprocal, and normalize into outT.  Runs 5
                    tiles into the next block so nothing on PE ever waits
                    for it.  The sums PSUM tile is drained to SBUF by a fast
                    DVE copy before the slow reciprocal reads it, so the
                    PSUM slot recycles in ~0.7us instead of ~6.5us (the
                    reciprocal-blocks-scores WAR stall)."""
                    h, pr = b // NPAIR, b % NPAIR
                    q0 = pr * 1024
                    acc_d, acc_p = acc_t.pop(b)
                    srcs = [acc_d, acc_p] + pt_tail.pop(b)
                    sums = sums_pool.tile(
                        [128, 1024], F32, tag=sums_tag, name="sums"
                    )
                    for hf in range(2):
                        hs = slice(hf * 512, (hf + 1) * 512)
                        for si, src in enumerate(srcs):
                            nc.tensor.matmul(
                                sums[:, hs], ones_bf, src[:, hs],
                                start=(si == 0), stop=(si == len(srcs) - 1),
                            )
                    ssb = smsb.tile([128, 1024], F32, tag="ssb", name="ssb")
                    nc.vector.tensor_copy(ssb, sums)
                    rc = rcp.tile([128, 1024], F32, tag="rc", name="rc")
                    nc.vector.reciprocal(rc, ssb)
                    # normalize on DVE right after the reciprocal so phase 3
                    # never waits behind the next block's Pool adds
                    nc.vector.tensor_tensor(
                        outT[:, h, q0 : q0 + 1024], avsb_t.pop(b), rc, Mult
                    )

                with tc.tile_pool(name="ps_mix", bufs=3, space="PSUM") as ps_mix:
                    for j in range(16 * NBLK + LOOK):
                        if j < 16 * NBLK:
                            b, tt = j // ST, j % ST
                            h, pr = b // NPAIR, b % NPAIR
                            q0 = pr * 1024
                            pss = ps_mix.tile(
                                [128, 1024], F32, tag="s", name="pss"
                            )
                            kslice = KT[:, tt * 128 : (tt + 1) * 128]
                            for hf in range(2):
                                nc.tensor.matmul(
                                    pss[:, hf * 512 : (hf + 1) * 512],
                                    kslice,
                                    QT[:, h, q0 + hf * 512 : q0 + (hf + 1) * 512],
                                    start=True, stop=True,
                                )
                            pss_t[j] = pss
                        jj = j - LOOK
                        if 0 <= jj < 16 * NBLK:
                            b, tt = jj // ST, jj % ST
                            h, pr = b // NPAIR, b % NPAIR
                            q0 = pr * 1024
                            pt = probs_pool.tile(
                                [128, 1024], BF16, tag="pt", name="pt"
                            )
                            nc.scalar.activation(pt, pss_t.pop(jj), Exp)
                            # denominator accumulation split, sized from
                            # measured per-op costs so each engine stays
                            # under PE's per-block time: DVE gets the init
                            # copy + 4 even tiles, Pool 6 tiles, PE the last
                            # 5 via finalize's ones-matmul
                            if tt == 0:
                                acc_d = accp.tile(
                                    [128, 1024], BF16, tag="accd", name="accd"
                                )
                                acc_p = accp.tile(
                                    [128, 1024], BF16, tag="accp", name="accp"
                                )
                                acc_t[b] = (acc_d, acc_p)
                                pt_tail[b] = []
                                nc.vector.tensor_copy(acc_d, pt)
                                nc.gpsimd.memset(acc_p, 0.0)
                                av_t[b] = ps_av.tile(
                                    [128, 1024], F32, tag="av", name="avp"
                                )
                            elif tt >= (ST - 7 if b == NBLK - 1 else ST - 5):
                                # last block hands two extra tiles to PE:
                                # there is no following block to hide the
                                # Pool adds' latency behind
                                pt_tail[b].append(pt)
                            elif tt % 2 == 0 and tt <= 8:
                                acc_d = acc_t[b][0]
                                nc.vector.tensor_tensor(acc_d, acc_d, pt, Add)
                            else:
                                acc_p = acc_t[b][1]
                                nc.gpsimd.tensor_tensor(acc_p, acc_p, pt, Add)
                            av = av_t[b]
                            for hf in range(2):
                                hs = slice(hf * 512, (hf + 1) * 512)
                                nc.tensor.matmul(
                                    av[:, hs], V[:, tt], pt[:, hs],
                                    start=(tt == 0), stop=(tt == ST - 1),
                                )
                            if tt == ST - 1:
                                # drain AV psum immediately so the next
                                # block's AV start never waits on the
                                # normalize chain
                                asb = avsb.tile(
                                    [128, 1024], F32, tag="avsb", name="asb"
                                )
                                avsb_t[b] = asb
                                nc.vector.tensor_copy(asb, av_t.pop(b))
                        # per-block tail emitted after this iteration's
                        # scores so the exp pipeline is never starved by
                        # the 14-matmul sums burst
                        if j >= 21 and (j - 21) % 16 == 0 and (j - 21) // 16 < NBLK - 1:
                            finalize((j - 21) // 16, ps_mix, "s")

                    # last block's reduction goes through the AV pool (not
                    # ps_mix), emitted before ps_mix closes so the close and
                    # phase 3's pool open overlap the final reciprocal
                    finalize(NBLK - 1, ps_av, "av")

                # ---- phase 3: output projection (transposed) ----
                otr = otd.rearrange("(o p) m -> p o m", p=128)
                with tc.tile_pool(name="ostage", bufs=4) as ostage, \
                     tc.tile_pool(name="ps_o", bufs=6, space="PSUM") as ps_o:
                    # sc outer: the first tiles only need the pr=0 (even)
                    # blocks, so P3 never waits on the last blocks' deferred
                    # normalize; output DMAs go out in et-pairs alternating
                    # between two DGE rings so the ring never backs up
                    for sc in range(SC):
                        cs = slice(sc * 512, (sc + 1) * 512)
                        for etp in range(ET // 2):
                            st = ostage.tile(
                                [128, 2, 512], F32, tag="ost", name="st"
                            )
                            for k in range(2):
                                et = etp * 2 + k
                                ps = ps_o.tile(
                                    [128, 512], F32, tag="po", name="ps"
                                )
                                for h in range(R):
                                    nc.tensor.matmul(
                                        ps,
                                        wo_sb[:, h, et * 128 : (et + 1) * 128],
                                        outT[:, h, cs],
                                        start=(h == 0), stop=(h == R - 1),
                                    )
                                nc.vector.tensor_copy(st[:, k], ps)
                            eng = (nc.sync, nc.scalar, nc.gpsimd)[etp % 3]
                            eng.dma_start(
                                otr[:, etp * 2 : etp * 2 + 2, cs], st
                            )

    _split_multi_waits(nc)
    return nc


def _prepare(x, Wq, bq, Wk, bk, Wv, bv, Wo, bo):
    """Host-side sharding: build per-core input maps."""
    x = np.asarray(x, dtype=np.float32)
    Wq = np.asarray(Wq, dtype=np.float32)
    bq = np.asarray(bq, dtype=np.float32)
    Wk = np.asarray(Wk, dtype=np.float32)
    bk = np.asarray(bk, dtype=np.float32)
    Wv = np.asarray(Wv, dtype=np.float32)
    bv = np.asarray(bv, dtype=np.float32)
    Wo = np.asarray(Wo, dtype=np.float32)

    import ml_dtypes

    BF = ml_dtypes.bfloat16
    isd = np.float32(1.0 / np.sqrt(D))
    xTs = [np.ascontiguousarray(x[b].T.astype(BF)) for b in range(B)]
    in_maps = []
    for core in range(8):
        b, g = divmod(core, G)
        in_maps.append({
            "xT": xTs[b],
            "wq": np.ascontiguousarray(
                (Wq[:, g * R * D : (g + 1) * R * D] * isd).astype(BF)
            ),
            "wk": np.ascontiguousarray(Wk[:, g * D : (g + 1) * D].astype(BF)),
            "wv": np.ascontiguousarray(Wv[:, g * D : (g + 1) * D].astype(BF)),
            "wo": np.ascontiguousarray(Wo[g * R * D : (g + 1) * R * D, :]),
            "bqv": bq[g * R * D : (g + 1) * R * D] * isd,
            "bkv": bk[g * D : (g + 1) * D],
            "bvv": bv[g * D : (g + 1) * D],
        })
    return in_maps


def _gather(results, bo):
    bo = np.asarray(bo, dtype=np.float32)
    out = np.empty((B, S, E), dtype=np.float32)
    for b in range(B):
        acc = results[b * G]["ot"].copy()
        for g in range(1, G):
            acc += results[b * G + g]["ot"]
        out[b] = acc.T + bo
    return out


def kernel(x, Wq, bq, Wk, bk, Wv, bv, Wo, bo):
    from concourse.bass_utils import run_bass_kernel_spmd

    if "nc" not in _cache:
        _cache["nc"] = _build_program()
    nc = _cache["nc"]
    in_maps = _prepare(x, Wq, bq, Wk, bk, Wv, bv, Wo, bo)
    res = run_bass_kernel_spmd(nc, in_maps, core_ids=list(range(8)))
    return _gather(res.results, bo)
